# revision 1
# baseline (speedup 1.0000x reference)
"""Trainium2 Bass kernel for nn_CausalSelfAttention_78331613544603.

Tensor-parallel over heads across 8 NeuronCores (Megatron-style):
each core computes QKV for its 2 heads, runs causal attention for its
(batch, head) pairs, projects with its w_proj column-slice, and the
partial outputs are combined with chunked ReduceScatter collectives.
The host shards the weights and gathers the output shards.

Self-contained: only needs numpy + the concourse toolchain staged at
/opt/trn_rl_repo (also importable via the environment's PYTHONPATH).
"""

import math
import sys

import numpy as np

try:
    import concourse.bass as bass
except ImportError:
    sys.path.insert(0, "/opt/trn_rl_repo")
    import concourse.bass as bass

import concourse.mybir as mybir
import concourse.tile as tile
from concourse import bacc, bass_utils

F32 = mybir.dt.float32
F32R = mybir.dt.float32r
BF16 = mybir.dt.bfloat16

N_CORES = 8
HEADS = 16
HPC = HEADS // N_CORES  # heads per core = 2
HD = 256  # head dim
KV_CHANNELS = 128
NEG = -1.0e30


class Cfg:
    def __init__(self, seq=2048, e=4096, out=2048):
        self.seq = seq  # sequence length
        self.batch = 2
        self.e = e  # input embedding dim (2*HIDDEN)
        self.out = out  # output dim (HIDDEN)
        self.ech = e // 128  # contraction chunks
        self.tok = seq * self.batch  # total tokens (batch-major)
        self.ntb = self.tok // 256  # qkv token blocks
        self.supers = seq // 256  # q super-tiles per (b,h)
        self.f_qk = HPC * HD * 2  # 1024 local q+k features
        self.f_v = HPC * HD  # 512 local v features
        self.nstg = self.tok // 512  # reduce-scatter chunks


def build_kernel(cfg: Cfg, debug_dumps=False):
    nc = bacc.Bacc("TRN2", target_bir_lowering=False, debug=False,
                   num_devices=N_CORES)

    ECH = cfg.ech
    SEQ = cfg.seq
    TOK = cfg.tok
    OUT = cfg.out

    # ---- kernel I/O ----
    hs5 = nc.dram_tensor("hs5", [cfg.ntb, ECH, 128, 256], F32R,
                         kind="ExternalInput")
    wqk = nc.dram_tensor("wqk", [ECH, 128, cfg.f_qk], F32R,
                         kind="ExternalInput")
    wv = nc.dram_tensor("wv", [ECH, 128, cfg.f_v], F32R, kind="ExternalInput")
    wp = nc.dram_tensor("wp", [4, 128, OUT], F32R, kind="ExternalInput")
    maskm = nc.dram_tensor("maskm", [128, 1024], F32, kind="ExternalInput")
    ident = nc.dram_tensor("ident", [128, 128], BF16, kind="ExternalInput")
    out_ext = nc.dram_tensor("out_ext", [cfg.nstg, 512 // N_CORES, OUT], F32,
                             kind="ExternalOutput")
    if debug_dumps:
        qk_dump = nc.dram_tensor("qk_dump", [8, 128, TOK], F32,
                                 kind="ExternalOutput")
        v_dump = nc.dram_tensor("v_dump", [128, TOK // 128, cfg.f_v], F32,
                                kind="ExternalOutput")
        yt_dump = nc.dram_tensor("yt_dump", [128, 4, TOK], F32,
                                 kind="ExternalOutput")
        part_dump = nc.dram_tensor("part_dump", [TOK, OUT], F32,
                                   kind="ExternalOutput")

    with tile.TileContext(nc) as tc:
        with (
            tc.tile_pool(name="const", bufs=1) as constp,
            tc.tile_pool(name="resident", bufs=1) as resp,
            tc.tile_pool(name="dram", bufs=1, space="DRAM") as dramp,
        ):
            mask_sb = constp.tile([128, 1024], F32, name="mask_sb")
            nc.sync.dma_start(mask_sb[:], maskm.ap())
            ident_sb = constp.tile([128, 128], BF16, name="ident_sb")
            nc.sync.dma_start(ident_sb[:], ident.ap())

            # v for all tokens stays resident through attention
            v_all = resp.tile([128, TOK // 128, cfg.f_v], BF16, name="v_all")

            qk_spill = dramp.tile([8, 128, TOK], F32, name="qk_spill")
            partial_c = [dramp.tile([512, OUT], F32, name=f"partial{c}",
                                    tag=f"partial{c}")
                         for c in range(cfg.nstg)]
            rs_out_c = [dramp.tile([512 // N_CORES, OUT], F32,
                                   name=f"rs_out{c}", tag=f"rs_out{c}")
                        for c in range(cfg.nstg)]

            # ================= phase 1: QK projection =================
            with (
                nc.named_scope("qk_proj"),
                tc.tile_pool(name="p1", bufs=1) as p1,
                tc.tile_pool(name="p1hs", bufs=2) as p1hs,
                tc.tile_pool(name="p1st", bufs=4) as p1st,
                tc.tile_pool(name="ps1", bufs=1, space="PSUM") as ps1,
            ):
                wqk_sb = {}
                for j in range(4):
                    for eh in range(2):
                        wqk_sb[(j, eh)] = p1.tile(
                            [128, ECH // 2, 256], F32R,
                            name=f"wqk_sb{j}_{eh}", tag=f"wqk_sb{j}_{eh}")

                def load_w(j, eh):
                    nc.sync.dma_start(
                        wqk_sb[(j, eh)][:],
                        wqk.ap()[eh * (ECH // 2):(eh + 1) * (ECH // 2), :,
                                 j * 256:(j + 1) * 256]
                        .rearrange("ec p f -> p ec f"),
                    )

                for tb in range(cfg.ntb):
                    pst = [
                        ps1.tile([128, 256], F32, name=f"ps_qk{fc}",
                                 tag=f"ps_qk{fc}")
                        for fc in range(8)
                    ]
                    for eh in range(2):
                        hs_t = p1hs.tile([128, ECH // 2, 256], F32R,
                                         name="hs_t", tag="hs_t")
                        nc.sync.dma_start(
                            hs_t[:],
                            hs5.ap()[tb, eh * (ECH // 2):(eh + 1) * (ECH // 2)]
                            .rearrange("ec p t -> p ec t"),
                        )
                        for j in range(4):
                            if tb == 0:
                                load_w(j, eh)
                            for el in range(ECH // 2):
                                ec = eh * (ECH // 2) + el
                                for half in range(2):
                                    fc = 2 * j + half
                                    nc.tensor.matmul(
                                        pst[fc][:],
                                        wqk_sb[(j, eh)][:, el,
                                                        half * 128:
                                                        (half + 1) * 128],
                                        hs_t[:, el, :],
                                        start=(ec == 0),
                                        stop=(ec == ECH - 1),
                                    )
                    for fc in range(8):
                            stg = p1st.tile([128, 256], F32, name="qkstg",
                                            tag="qkstg")
                            nc.vector.tensor_copy(stg[:], pst[fc][:])
                            nc.sync.dma_start(
                                qk_spill[fc, :, tb * 256:(tb + 1) * 256],
                                stg[:])
                            if debug_dumps:
                                nc.sync.dma_start(
                                    qk_dump.ap()[fc, :,
                                                 tb * 256:(tb + 1) * 256],
                                    stg[:])

            # ================= phase 2: V projection =================
            with (
                nc.named_scope("v_proj"),
                tc.tile_pool(name="p2", bufs=1) as p2,
                tc.tile_pool(name="p2hs", bufs=2) as p2hs,
                tc.tile_pool(name="ps2", bufs=3, space="PSUM") as ps2,
            ):
                wv_sb = p2.tile([128, ECH, cfg.f_v], F32R, name="wv_sb")
                nc.gpsimd.dma_start(wv_sb[:],
                                    wv.ap().rearrange("ec p f -> p ec f"))
                for tb in range(cfg.ntb):
                    for eh in range(2):
                        hs_t2 = p2hs.tile([128, ECH // 2, 256], F32R,
                                          name="hs_t2", tag="hs_t2")
                        nc.sync.dma_start(
                            hs_t2[:],
                            hs5.ap()[tb, eh * (ECH // 2):(eh + 1) * (ECH // 2)]
                            .rearrange("ec p t -> p ec t"),
                        )
                        if eh == 0:
                            ps_v = [
                                ps2.tile([128, cfg.f_v], F32, name="ps_v",
                                         tag="ps_v")
                                for _ in range(2)
                            ]
                        for tc_i in range(2):
                            for el in range(ECH // 2):
                                ec = eh * (ECH // 2) + el
                                nc.tensor.matmul(
                                    ps_v[tc_i][:],
                                    hs_t2[:, el, tc_i * 128:(tc_i + 1) * 128],
                                    wv_sb[:, ec, :],
                                    start=(ec == 0),
                                    stop=(ec == ECH - 1),
                                )
                    for tc_i in range(2):
                        g = tb * 2 + tc_i
                        nc.vector.tensor_copy(v_all[:, g, :], ps_v[tc_i][:])
                if debug_dumps:
                    vstg = p2.tile([128, TOK // 128, cfg.f_v], F32,
                                   name="vstg")
                    nc.vector.tensor_copy(vstg[:], v_all[:])
                    nc.sync.dma_start(v_dump.ap(), vstg[:])

            # y^T per batch, alive from attention through the projection
            ytp_cm = tc.tile_pool(name="ytp", bufs=1)
            ytp = ytp_cm.__enter__()
            nstg_b = cfg.nstg // cfg.batch
            yt_t = {}
            for b in range(cfg.batch):
                for sl in range(nstg_b):
                    yt_t[(b, sl)] = ytp.tile(
                        [128, 4, 512], F32R, name=f"yt_{b}_{sl}",
                        tag=f"yt_{b}_{sl}")

            # ====== phases 3+4: attention software-pipelined with =========
            # ====== per-batch output projection + ReduceScatter    =========
            with (
                nc.named_scope("attn_proj"),
                tc.tile_pool(name="qkp", bufs=2) as qkp,
                tc.tile_pool(name="qsp", bufs=2) as qsp,
                tc.tile_pool(name="strips", bufs=2) as strips,
                tc.tile_pool(name="ptp", bufs=2) as ptp,
                tc.tile_pool(name="statp", bufs=4) as statp,
                tc.tile_pool(name="wpp", bufs=2) as wpp,
                tc.tile_pool(name="pstrips", bufs=2) as pstrips,
                tc.tile_pool(name="p4st", bufs=2) as p4st,
                tc.tile_pool(name="ps3", bufs=2, space="PSUM") as ps3,
                tc.tile_pool(name="ps3b", bufs=1, space="PSUM") as ps3b,
                tc.tile_pool(name="ps3o", bufs=2, space="PSUM") as ps3o,
            ):
                n_ob = OUT // 512
                kp_tiles = {}

                def produce(b, h, u):
                    """QK for one q-super: S blocks -> masked strips."""
                    if u == 0:
                        kp = qkp.tile([128, 2, SEQ], F32, name="kp", tag="kp")
                        for dc in range(2):
                            nc.sync.dma_start(
                                kp[:, dc, :],
                                qk_spill[4 + 2 * h + dc, :,
                                         b * SEQ:(b + 1) * SEQ])
                        kp_tiles[(b, h)] = kp
                    kp = kp_tiles[(b, h)]
                    qs = qsp.tile([128, 2, 256], F32, name="qs", tag="qs")
                    for dc in range(2):
                        nc.sync.dma_start(
                            qs[:, dc, :],
                            qk_spill[2 * h + dc, :,
                                     b * SEQ + u * 256:b * SEQ + (u + 1) * 256])
                    nb = (u + 2) // 2
                    # PV only reads cols [0, (2u+2)*128); trim the last
                    # block to 256 wide when nb*512 overshoots by 256
                    rem = nb * 512 - (2 * u + 2) * 128
                    strip = [
                        strips.tile([128, SEQ], F32, name=f"strip{qt}",
                                    tag=f"strip{qt}")
                        for qt in range(2)
                    ]
                    for qt in range(2):
                        i = 2 * u + qt
                        d_jb = i // 4
                        for jb in range(nb):
                            w = 512 - rem if jb == nb - 1 else 512
                            dst = strip[qt][:, jb * 512:jb * 512 + w]
                            if jb > d_jb:
                                nc.scalar.copy(dst, mask_sb[:, 512:512 + w])
                                continue
                            ps_s = ps3.tile([128, 512], F32,
                                            name="ps_s", tag="ps_s")
                            for ec in range(2):
                                nc.tensor.matmul(
                                    ps_s[:, :w],
                                    qs[:, ec, qt * 128:(qt + 1) * 128],
                                    kp[:, ec, jb * 512:jb * 512 + w],
                                    start=(ec == 0),
                                    stop=(ec == 1),
                                )
                            if jb == d_jb:
                                off = 384 - 128 * (i % 4)
                                nc.vector.tensor_tensor(
                                    dst, ps_s[:, :w],
                                    mask_sb[:, off:off + w],
                                    mybir.AluOpType.add)
                            else:
                                nc.scalar.copy(dst, ps_s[:, :w])
                    return strip

                def consume(b, h, u, strip):
                    """softmax -> transpose P -> PV -> y^T drain."""
                    nb = (u + 2) // 2
                    nk = 2 * (u + 1)
                    Lp = nk * 128  # range PV reads (exp'd, masked -> 0)
                    pstrip = [
                        pstrips.tile([128, SEQ], BF16, name=f"pstrip{qt}",
                                     tag=f"pstrip{qt}")
                        for qt in range(2)
                    ]
                    for qt in range(2):
                        Lv = (2 * u + qt + 1) * 128  # causally valid cols
                        negmax = statp.tile([128, 1], F32, name="negmax",
                                            tag="negmax")
                        nc.vector.reduce_max(
                            negmax[:], strip[qt][:, :Lv],
                            axis=mybir.AxisListType.X, negate=True)
                        zsum = statp.tile([128, 1], F32, name="zsum",
                                          tag="zsum")
                        nc.scalar.activation(
                            pstrip[qt][:, :Lp], strip[qt][:, :Lp],
                            mybir.ActivationFunctionType.Exp,
                            bias=negmax[:], scale=1.0, accum_out=zsum[:])
                        rz = statp.tile([128, 1], F32, name="rz", tag="rz")
                        nc.vector.reciprocal(rz[:], zsum[:])
                        nc.vector.tensor_scalar_mul(
                            pstrip[qt][:, :Lp], pstrip[qt][:, :Lp], rz[:])
                    ps_y = [
                        ps3b.tile([128, 256], F32, name=f"ps_y{dh}",
                                  tag=f"ps_y{dh}")
                        for dh in range(2)
                    ]
                    for c in range(nk):
                        pt_sb = ptp.tile([128, 256], BF16, name="pt_sb",
                                         tag="pt_sb")
                        for qt in range(2):
                            ps_pt = ps3.tile([128, 128], BF16,
                                             name="ps_pt", tag="ps_pt")
                            nc.tensor.transpose(
                                ps_pt[:],
                                pstrip[qt][:, c * 128:(c + 1) * 128],
                                ident_sb[:])
                            nc.vector.tensor_copy(
                                pt_sb[:, qt * 128:(qt + 1) * 128], ps_pt[:])
                        g = b * (SEQ // 128) + c
                        for dh in range(2):
                            nc.tensor.matmul(
                                ps_y[dh][:],
                                v_all[:, g, h * 256 + dh * 128:
                                      h * 256 + (dh + 1) * 128],
                                pt_sb[:],
                                start=(c == 0),
                                stop=(c == nk - 1),
                            )
                    for dh in range(2):
                        nc.scalar.copy(
                            yt_t[(b, u // 2)][:, 2 * h + dh,
                                              (u % 2) * 256:
                                              (u % 2 + 1) * 256],
                            ps_y[dh][:])

                def proj_rs(b, sl):
                    """project one 512-token chunk of y^T, reduce-scatter."""
                    stg = b * nstg_b + sl
                    for ob in range(n_ob):
                        wpt = wpp.tile([128, 4, 512], F32R, name="wpt",
                                       tag="wpt")
                        for fc in range(4):
                            nc.sync.dma_start(
                                wpt[:, fc, :],
                                wp.ap()[fc, :, ob * 512:(ob + 1) * 512])
                        for st in range(4):
                            t0 = st * 128
                            ps_o = ps3o.tile([128, 512], F32, name="ps_o",
                                             tag="ps_o")
                            for fc in range(4):
                                nc.tensor.matmul(
                                    ps_o[:],
                                    yt_t[(b, sl)][:, fc, t0:t0 + 128],
                                    wpt[:, fc, :],
                                    start=(fc == 0),
                                    stop=(fc == 3),
                                )
                            ost = p4st.tile([128, 512], F32, name="ost",
                                            tag="ost")
                            nc.vector.tensor_copy(ost[:], ps_o[:])
                            nc.sync.dma_start(
                                partial_c[stg][t0:t0 + 128,
                                               ob * 512:(ob + 1) * 512],
                                ost[:])
                            if debug_dumps:
                                gt0 = stg * 512 + t0
                                nc.sync.dma_start(
                                    part_dump.ap()[gt0:gt0 + 128,
                                                   ob * 512:(ob + 1) * 512],
                                    ost[:])
                    nc.gpsimd.collective_compute(
                        "ReduceScatter",
                        mybir.AluOpType.add,
                        ins=[partial_c[stg].opt()],
                        outs=[rs_out_c[stg].opt()],
                        replica_groups=[list(range(N_CORES))],
                    )
                    nc.gpsimd.dma_start(out_ext.ap()[stg], rs_out_c[stg])

                stages = [(b, h, u)
                          for b in range(cfg.batch)
                          for h in range(HPC)
                          for u in range(cfg.supers)]
                def after_consume(pb, ph, pu):
                    if ph == HPC - 1 and pu % 2 == 1:
                        sl = pu // 2
                        if debug_dumps:
                            g0 = pb * SEQ + sl * 512
                            nc.sync.dma_start(
                                yt_dump.ap()[:, :, g0:g0 + 512],
                                yt_t[(pb, sl)][:].bitcast(F32))
                        proj_rs(pb, sl)

                prev = None
                for stage in stages:
                    strip = produce(*stage)
                    if prev is not None:
                        consume(*prev[0], prev[1])
                        after_consume(*prev[0])
                    prev = (stage, strip)
                consume(*prev[0], prev[1])
                after_consume(*prev[0])
            ytp_cm.__exit__(None, None, None)

    nc.finalize()
    return nc


def prep_inputs(cfg: Cfg, hidden_states, w_qkv, w_proj):
    """Shard + lay out the full inputs for each of the 8 cores."""
    seq, batch, e = hidden_states.shape
    assert (seq, batch, e) == (cfg.seq, cfg.batch, cfg.e)
    hs_t = np.ascontiguousarray(
        hidden_states.transpose(1, 0, 2).reshape(cfg.tok, e).T
    )  # [e, tok], tokens batch-major
    # [ntb, ech, 128, 256]
    hs5 = np.ascontiguousarray(
        hs_t.reshape(cfg.ech, 128, cfg.ntb, 256).transpose(2, 0, 1, 3)
    ).astype(np.float32)

    scale = math.sqrt(math.sqrt(KV_CHANNELS))
    w3 = w_qkv.reshape(HEADS, 3, HD, e)
    mask = np.full((128, 1024), 0.0, dtype=np.float32)
    cols = np.arange(1024)[None, :]
    rows = np.arange(128)[:, None]
    mask[cols > 384 + rows] = NEG
    import ml_dtypes
    ident = np.eye(128, dtype=ml_dtypes.bfloat16)

    in_maps = []
    for c in range(N_CORES):
        hsel = [2 * c, 2 * c + 1]
        w_q = (w3[hsel, 0] * scale).reshape(cfg.f_qk // 2, e)
        w_k = (w3[hsel, 1] * scale).reshape(cfg.f_qk // 2, e)
        w_v = w3[hsel, 2].reshape(cfg.f_v, e)
        wqk = np.concatenate([w_q, w_k], axis=0)  # [1024, e]
        wqk_t = np.ascontiguousarray(wqk.T.reshape(cfg.ech, 128, cfg.f_qk))
        wv_t = np.ascontiguousarray(w_v.T.reshape(cfg.ech, 128, cfg.f_v))
        wp_c = w_proj[:, c * cfg.f_v:(c + 1) * cfg.f_v]  # [OUT, 512]
        wp_t = np.ascontiguousarray(wp_c.T.reshape(4, 128, cfg.out))
        in_maps.append({
            "hs5": hs5,
            "wqk": wqk_t.astype(np.float32),
            "wv": wv_t.astype(np.float32),
            "wp": wp_t.astype(np.float32),
            "maskm": mask,
            "ident": ident,
        })
    return in_maps


def assemble_output(cfg: Cfg, results):
    """Gather per-core ReduceScatter shards into the full [seq, b, out]."""
    rows = 512 // N_CORES
    full = np.empty((cfg.tok, cfg.out), dtype=np.float32)
    for stg in range(cfg.nstg):
        for r in range(N_CORES):
            t0 = stg * 512 + r * rows
            full[t0:t0 + rows] = results[r]["out_ext"][stg]
    return np.ascontiguousarray(
        full.reshape(cfg.batch, cfg.seq, cfg.out).transpose(1, 0, 2))


_NC_CACHE = {}


def run(cfg: Cfg, hidden_states, w_qkv, w_proj, trace=False):
    key = (cfg.seq, cfg.e, cfg.out)
    if key not in _NC_CACHE:
        _NC_CACHE[key] = build_kernel(cfg)
    nc = _NC_CACHE[key]
    in_maps = prep_inputs(cfg, hidden_states, w_qkv, w_proj)
    res = bass_utils.run_bass_kernel_spmd(
        nc, in_maps, core_ids=list(range(N_CORES)), trace=trace)
    return assemble_output(cfg, res.results), res


def kernel(hidden_states, attention_mask, w_qkv, w_proj):
    cfg = Cfg()
    out, _ = run(cfg, np.asarray(hidden_states, dtype=np.float32),
                 np.asarray(w_qkv, dtype=np.float32),
                 np.asarray(w_proj, dtype=np.float32))
    return out



# revision 27
# speedup vs baseline: 1.1400x; 1.1400x over previous
"""Trainium2 Bass kernel for nn_CausalSelfAttention_78331613544603.

Tensor-parallel over heads across 8 NeuronCores (Megatron-style).
Per core (2 heads), three software-pipelined phases:
  1a: QK projection for head0 + V projection for both heads (bf16).
  1b: QK projection for head1 on the PE, interleaved with causal
      attention for batch0 (both heads), batch0 output projection and
      its chunked ReduceScatter.
  2b: attention for batch1 + output projection + ReduceScatter.
Scores run in fp32r (1 cycle/row); P^T is produced by DMA transpose
(xbar) instead of PE transposes; partial sums reduce in bf16.

Self-contained: only needs numpy + the concourse toolchain staged at
/opt/trn_rl_repo (also importable via the environment's PYTHONPATH).
"""

import math
import sys

import numpy as np

try:
    import concourse.bass as bass
except ImportError:
    sys.path.insert(0, "/opt/trn_rl_repo")
    import concourse.bass as bass

import concourse.mybir as mybir
import concourse.tile as tile
from concourse import bacc, bass_utils

F32 = mybir.dt.float32
F32R = mybir.dt.float32r
BF16 = mybir.dt.bfloat16
FP16 = mybir.dt.float16
FP8 = mybir.dt.float8e4
LOSC = 32.0

N_CORES = 8
HEADS = 16
HPC = HEADS // N_CORES  # heads per core = 2
HD = 256  # head dim
KV_CHANNELS = 128
NEG = -1.0e30


def _copy(nc, eng, dst, src):
    if eng is nc.scalar:
        nc.scalar.copy(dst, src)
    else:
        eng.tensor_copy(dst, src)


# RS chunks: (token_start, n_tokens, batch, [(sl, st0, nst), ...]).
# b0 chunks run as phase-2b fillers; b1 chunks fire as their supers
# complete (2b processes supers in descending order, so high sls first;
# the last two chunks are 256 tokens to shrink the collective tail).
CHUNKS = [
    (0, 1024, 0, [(0, 0, 4), (1, 0, 4)]),
    (1024, 1024, 0, [(2, 0, 4), (3, 0, 4)]),
    (2048, 512, 1, [(0, 0, 4)]),
    (2560, 512, 1, [(1, 0, 4)]),
    (3072, 512, 1, [(2, 0, 4)]),
    (3584, 256, 1, [(3, 0, 2)]),
    (3840, 256, 1, [(3, 2, 2)]),
]


def chunk_reqs(ci):
    t0, nt, b, parts = CHUNKS[ci]
    req = set()
    for (sl, st0, nst) in parts:
        for st in range(st0, st0 + nst):
            u = 2 * sl + (1 if st >= 2 else 0)
            for h in range(2):
                req.add((b, h, u))
    return req


class Cfg:
    def __init__(self, seq=2048, e=4096, out=2048):
        self.seq = seq
        self.batch = 2
        self.e = e  # input embedding dim (2*HIDDEN)
        self.out = out  # output dim (HIDDEN)
        self.ech = e // 128  # contraction chunks (32)
        self.tok = seq * self.batch  # total tokens, batch-major (4096)
        self.ntb = self.tok // 256  # projection token blocks (16)
        self.supers = seq // 256  # q super-tiles per (b,h) (8)
        self.nstg = self.tok // 512  # reduce-scatter chunks (8)


def build_kernel(cfg: Cfg):
    nc = bacc.Bacc("TRN2", target_bir_lowering=False, debug=False,
                   num_devices=N_CORES)

    ECH = cfg.ech  # 32
    SEQ = cfg.seq
    TOK = cfg.tok
    OUT = cfg.out
    NTB = cfg.ntb
    SUP = cfg.supers

    # ---- kernel I/O ----
    hs5 = nc.dram_tensor("hs5", [NTB, ECH, 128, 256], F32R,
                         kind="ExternalInput")
    wqk = nc.dram_tensor("wqk", [ECH, 128, 1024], F32R,
                         kind="ExternalInput")  # head-major: h*512+(q,k)
    wv = nc.dram_tensor("wv", [ECH, 128, 512], BF16, kind="ExternalInput")
    wp = nc.dram_tensor("wp", [4, 128, OUT], BF16, kind="ExternalInput")
    maskm = nc.dram_tensor("maskm", [128, 384], F32, kind="ExternalInput")
    identm = nc.dram_tensor("identm", [128, 128], BF16, kind="ExternalInput")
    out_ext = nc.dram_tensor("out_ext", [cfg.tok // N_CORES, OUT],
                             BF16, kind="ExternalOutput")

    with tile.TileContext(nc) as tc:
        with (
            tc.tile_pool(name="const", bufs=1) as constp,
            tc.tile_pool(name="resident", bufs=1) as resp,
            tc.tile_pool(name="dram", bufs=1, space="DRAM") as dramp,
        ):
            mask_sb = constp.tile([128, 384], F32, name="mask_sb")
            nc.sync.dma_start(mask_sb[:], maskm.ap())
            ident_holder = {}

            # v for all tokens / both heads, resident through attention
            v_all = resp.tile([128, TOK // 128, 512], BF16, name="v_all")
            # y^T accumulators: 4 slots, reused across batches
            yt_t = {}

            def get_yt(b, sl):
                if (b, sl) not in yt_t:
                    yt_t[(b, sl)] = resp.tile(
                        [128, 4, 512], BF16, name=f"yt_{b}_{sl}",
                        tag=f"yt_{sl}")
                return yt_t[(b, sl)]

            qh_spill = dramp.tile([2, 128, NTB, 2, 256], FP16,
                                  name="qh_spill")
            kh_spill = dramp.tile([2, 128, NTB, 2, 256], FP16,
                                  name="kh_spill")
            q8_spill = dramp.tile([2, 128, NTB, 2, 2, 256], FP8,
                                  name="q8_spill")
            k8_spill = dramp.tile([2, 128, NTB, 2, 2, 256], FP8,
                                  name="k8_spill")
            partial_c = [dramp.tile([nt, OUT], BF16, name=f"partial{ci}",
                                    tag=f"partial{ci}")
                         for ci, (t0, nt, b, parts) in enumerate(CHUNKS)]
            rs_out_c = [dramp.tile([nt // N_CORES, OUT], BF16,
                                   name=f"rs_out{ci}", tag=f"rs_out{ci}")
                        for ci, (t0, nt, b, parts) in enumerate(CHUNKS)]

            # ====== shared attention machinery (used by 1b and 2b) ======
            def make_attn_pools(stack, with_proj_out):
                pools = {}
                pools["kp"] = stack.enter_context(
                    tc.tile_pool(name="kp", bufs=1))
                pools["qs"] = stack.enter_context(
                    tc.tile_pool(name="qs", bufs=3))
                pools["strip"] = stack.enter_context(
                    tc.tile_pool(name="strip", bufs=3))
                pools["pstrip"] = stack.enter_context(
                    tc.tile_pool(name="pstrip", bufs=3))
                pools["pt"] = stack.enter_context(
                    tc.tile_pool(name="pt", bufs=2))
                pools["stat"] = stack.enter_context(
                    tc.tile_pool(name="stat", bufs=2))
                pools["ps_s"] = stack.enter_context(
                    tc.tile_pool(name="ps_s", bufs=2, space="PSUM"))
                pools["ps_y"] = stack.enter_context(
                    tc.tile_pool(name="ps_y", bufs=1, space="PSUM"))
                if with_proj_out:
                    add_proj_out_pools(stack, pools)
                return pools

            def add_proj_out_pools(stack, pools):
                pools["ps_pt"] = stack.enter_context(
                    tc.tile_pool(name="ps_pt", bufs=2, space="PSUM"))
                pools["wpp"] = stack.enter_context(
                    tc.tile_pool(name="wpp", bufs=1))
                ident_sb = pools["wpp"].tile([128, 128], BF16,
                                             name="ident_sb")
                nc.sync.dma_start(ident_sb[:], identm.ap())
                ident_holder["t"] = ident_sb
                pools["ost"] = stack.enter_context(
                    tc.tile_pool(name="ost", bufs=2))
                pools["ps_o"] = stack.enter_context(
                    tc.tile_pool(name="ps_o", bufs=2, space="PSUM"))
                wp_sb = pools["wpp"].tile([128, 4, OUT], BF16, name="wp_sb")
                for ci in range(2):
                    nc.scalar.dma_start(
                        wp_sb[:, 2 * ci:2 * ci + 2, :],
                        wp.ap()[2 * ci:2 * ci + 2]
                        .rearrange("fc p o -> p fc o"))
                pools["wp_sb"] = wp_sb

            kp_tiles = {}
            qs_tiles = {}
            pt_tiles = {}

            def prefetch_kp(pools, b, h, u):
                """Load K increment j=u: fp16 hi + fp8 cross pieces."""
                nb = 2 if pools.get("kp_bufs", 1) > 1 else 1
                kp_t = pools["kp"].tile([128, 2, 256], FP16, name="kph",
                                        tag=f"kph{u}", bufs=nb)
                pools["dma_q"].dma_start(kp_t[:], kh_spill[h, :, b * 8 + u])
                kp8_t = pools["kp"].tile([128, 2, 2, 256], FP8, name="kp8",
                                         tag=f"kp8{u}", bufs=nb)
                pools["dma_q"].dma_start(kp8_t[:], k8_spill[h, :, b * 8 + u])
                kp_tiles[(b, h, u)] = (kp_t, kp8_t)

            def prefetch_qs(pools, b, h, u):
                qs_t = pools["qs"].tile([128, 2, 256], FP16, name="qsh",
                                        tag="qsh", bufs=2)
                pools["dma_q"].dma_start(qs_t[:], qh_spill[h, :, b * 8 + u])
                qs8_t = pools["qs"].tile([128, 2, 2, 256], FP8, name="qs8",
                                         tag="qs8", bufs=2)
                pools["dma_q"].dma_start(qs8_t[:], q8_spill[h, :, b * 8 + u])
                qs_tiles[(b, h, u)] = (qs_t, qs8_t)

            def produce(pools, b, h, u):
                """scores for super u -> softmax -> P^T via DMA transpose."""
                qs_t, qs8_t = qs_tiles.pop((b, h, u))
                nk = 2 * (u + 1)
                Lp = nk * 128
                pt_sb = pools["pt"].tile([128, SEQ // 128, 256], BF16,
                                         name="pt", tag="pt")
                pt_tiles[(b, h, u)] = pt_sb
                pstrips = {}
                for qt in range(2):
                    i = 2 * u + qt
                    strip = pools["strip"].tile([128, SEQ], F32,
                                                name="strip", tag="strip")
                    for j in range(u + 1):
                        ps_s = pools["ps_s"].tile([128, 256], F32,
                                                  name="ps_s", tag="ps_s")
                        kp_t, kp8_t = kp_tiles[(b, h, j)]
                        for ec in range(2):
                            nc.tensor.matmul(
                                ps_s[:],
                                qs_t[:, ec, qt * 128:(qt + 1) * 128],
                                kp_t[:, ec, :],
                                start=(ec == 0), stop=False)
                        for ec in range(2):
                            nc.tensor.matmul(
                                ps_s[:],
                                qs8_t[:, ec, :, qt * 128:(qt + 1) * 128],
                                kp8_t[:, ec, :, :],
                                perf_mode=mybir.MatmulPerfMode.DoubleRow,
                                start=False, stop=(ec == 1))
                        dst = strip[:, j * 256:(j + 1) * 256]
                        if j == u:
                            off = 128 if qt == 0 else 0
                            nc.vector.tensor_tensor(
                                dst, ps_s[:], mask_sb[:, off:off + 256],
                                mybir.AluOpType.add)
                        elif j % 2 == 0:
                            nc.scalar.copy(dst, ps_s[:])
                        else:
                            nc.vector.tensor_copy(dst, ps_s[:])
                    Lv = (i + 1) * 128
                    negmax = pools["stat"].tile([128, 1], F32, name="negmax",
                                                tag="negmax")
                    nc.vector.reduce_max(negmax[:], strip[:, :Lv],
                                         axis=mybir.AxisListType.X,
                                         negate=True)
                    pstrip = pools["pstrip"].tile([128, SEQ], BF16,
                                                  name="pstrip", tag="pstrip")
                    zsum = pools["stat"].tile([128, 1], F32, name="zsum",
                                              tag="zsum")
                    nc.scalar.activation(
                        pstrip[:, :Lp], strip[:, :Lp],
                        mybir.ActivationFunctionType.Exp,
                        bias=negmax[:], scale=1.0, accum_out=zsum[:])
                    rz = pools["stat"].tile([128, 1], F32, name="rz",
                                            tag="rz")
                    nc.vector.reciprocal(rz[:], zsum[:])
                    nc.vector.tensor_scalar_mul(
                        pstrip[:, :Lp], pstrip[:, :Lp], rz[:])
                    if "ps_pt" in pools:
                        pstrips[qt] = pstrip
                    else:
                        pools["tq"].dma_start_transpose(
                            pt_sb[:, :nk, qt * 128:(qt + 1) * 128],
                            pstrip[:, :Lp])
                if "ps_pt" in pools:
                    for c in range(nk):
                        ps_pt = pools["ps_pt"].tile([128, 256], BF16,
                                                    name="ps_pt", tag="ps_pt")
                        for qt in range(2):
                            nc.tensor.transpose(
                                ps_pt[:, qt * 128:(qt + 1) * 128],
                                pstrips[qt][:, c * 128:(c + 1) * 128],
                                ident_holder["t"][:])
                        eng = nc.vector if c % 2 == 0 else nc.scalar
                        _copy(nc, eng, pt_sb[:, c, :], ps_pt[:])

            def consume(pools, b, h, u):
                """PV for super u -> y^T chunk."""
                nk = 2 * (u + 1)
                pt_sb = pt_tiles.pop((b, h, u))
                ps_y = [pools["ps_y"].tile([128, 256], F32, name=f"ps_y{dh}",
                                           tag=f"ps_y{dh}")
                        for dh in range(2)]
                for c in range(nk):
                    g = b * (SEQ // 128) + c
                    for dh in range(2):
                        nc.tensor.matmul(
                            ps_y[dh][:],
                            v_all[:, g, h * 256 + dh * 128:
                                  h * 256 + (dh + 1) * 128],
                            pt_sb[:, c, :],
                            start=(c == 0), stop=(c == nk - 1))
                yt = get_yt(b, u // 2)
                for dh in range(2):
                    eng = nc.vector if dh == 0 else nc.scalar
                    _copy(nc, eng,
                          yt[:, 2 * h + dh, (u % 2) * 256:(u % 2 + 1) * 256],
                          ps_y[dh][:])

            def proj_rs(pools, ci):
                """output projection for one RS chunk."""
                t0_tok, nt, b, parts = CHUNKS[ci]
                wp_sb = pools["wp_sb"]
                row = 0
                for (sl, st0, nst) in parts:
                    yt = yt_t[(b, sl)]
                    for st in range(st0, st0 + nst):
                        ost = pools["ost"].tile([128, OUT], BF16, name="ost",
                                                tag="ost")
                        for ob in range(OUT // 512):
                            ps_o = pools["ps_o"].tile([128, 512], F32,
                                                      name="ps_o", tag="ps_o")
                            for fc in range(4):
                                nc.tensor.matmul(
                                    ps_o[:],
                                    yt[:, fc, st * 128:(st + 1) * 128],
                                    wp_sb[:, fc, ob * 512:(ob + 1) * 512],
                                    start=(fc == 0), stop=(fc == 3))
                            eng = nc.vector if ob % 2 == 0 else nc.scalar
                            _copy(nc, eng, ost[:, ob * 512:(ob + 1) * 512],
                                  ps_o[:])
                        nc.sync.dma_start(partial_c[ci][row:row + 128, :],
                                          ost[:])
                        row += 128
                nc.gpsimd.collective_compute(
                    "ReduceScatter",
                    mybir.AluOpType.add,
                    ins=[partial_c[ci].opt()],
                    outs=[rs_out_c[ci].opt()],
                    replica_groups=[list(range(N_CORES))],
                )
                nc.gpsimd.dma_start(
                    out_ext.ap()[t0_tok // N_CORES:
                                 (t0_tok + nt) // N_CORES],
                    rs_out_c[ci])

            def drain_qk(stpool, pst, hidx, tb):
                """Drain 4 qk PSUM groups into fp16-hi + scaled-fp8 spills.
                q pieces: (lo*32, hi/32); k pieces: (hi/32, lo*32)."""
                for pair in range(2):
                    hi = stpool.tile([128, 2, 256], FP16, name="hi",
                                     tag="hi")
                    p8 = stpool.tile([128, 2, 2, 256], FP8, name="p8",
                                     tag="p8")
                    for half in range(2):
                        fc = pair * 2 + half
                        eng = nc.vector if half == 0 else nc.scalar
                        _copy(nc, eng, hi[:, half, :], pst[fc][:])
                        lo = stpool.tile([128, 256], FP16, name="lo",
                                         tag="lo")
                        nc.vector.tensor_tensor(lo[:], pst[fc][:],
                                                hi[:, half, :],
                                                mybir.AluOpType.subtract)
                        lo_pc = 0 if pair == 0 else 1
                        nc.gpsimd.tensor_scalar_mul(
                            p8[:, half, lo_pc, :], lo[:], LOSC)
                        nc.gpsimd.tensor_scalar_mul(
                            p8[:, half, 1 - lo_pc, :], hi[:, half, :],
                            1.0 / LOSC)
                    hdst = qh_spill if pair == 0 else kh_spill
                    dst8 = q8_spill if pair == 0 else k8_spill
                    nc.sync.dma_start(hdst[hidx, :, tb], hi[:])
                    nc.sync.dma_start(dst8[hidx, :, tb], p8[:])

            # ================= phase 1a: qk(h0) + v(both) =================
            with (
                nc.named_scope("proj_a"),
                tc.tile_pool(name="p1w", bufs=1) as p1w,
                tc.tile_pool(name="p1hs", bufs=3) as p1hs,
                tc.tile_pool(name="p1hb", bufs=3) as p1hb,
                tc.tile_pool(name="p1st", bufs=6) as p1st,
                tc.tile_pool(name="psA", bufs=1, space="PSUM") as psA,
                tc.tile_pool(name="psV", bufs=2, space="PSUM") as psV,
            ):
                wqk_sb = p1w.tile([128, ECH, 512], F32R, name="wqk0_sb")
                wv_sb = p1w.tile([128, ECH, 512], BF16, name="wv_sb")

                def load_wqk0(ci):
                    e0 = ci * (ECH // 4)
                    e1 = (ci + 1) * (ECH // 4)
                    nc.sync.dma_start(
                        wqk_sb[:, e0:e1, :],
                        wqk.ap()[e0:e1, :, 0:512].rearrange("ec p f -> p ec f"))
                    nc.scalar.dma_start(
                        wv_sb[:, e0:e1, :],
                        wv.ap()[e0:e1].rearrange("ec p f -> p ec f"))


                load_wqk0(0)
                for tb in range(NTB):
                    hs_eh = {}
                    hb_eh = {}

                    def load_group(g, tb=tb, hs_eh=hs_eh, hb_eh=hb_eh):
                        hs_t = p1hs.tile([128, 8, 256], F32R, name="hs_t",
                                         tag="hs_t")
                        nc.sync.dma_start(
                            hs_t[:],
                            hs5.ap()[tb, g * 8:(g + 1) * 8]
                            .rearrange("ec p t -> p ec t"))
                        hb_t = p1hb.tile([128, 8, 256], BF16, name="hb_t",
                                         tag="hb_t")
                        nc.gpsimd.tensor_copy(hb_t[:], hs_t[:])
                        hs_eh[g] = hs_t
                        hb_eh[g] = hb_t

                    load_group(0)
                    pst = [psA.tile([128, 256], F32, name=f"ps_qk{fc}",
                                    tag=f"ps_qk{fc}") for fc in range(4)]
                    ps_v = [psV.tile([128, 512], F32, name=f"ps_v{tc_i}",
                                     tag=f"ps_v{tc_i}") for tc_i in range(2)]

                    def emit_v(ec):
                        for tc_i in range(2):
                            nc.tensor.matmul(
                                ps_v[tc_i][:],
                                hb_eh[ec // 8][:, ec % 8,
                                               tc_i * 128:(tc_i + 1) * 128],
                                wv_sb[:, ec, :],
                                start=(ec == 0), stop=(ec == ECH - 1))

                    def emit_qk(ec):
                        for fc in range(4):
                            nc.tensor.matmul(
                                pst[fc][:],
                                wqk_sb[:, ec, fc * 128:(fc + 1) * 128],
                                hs_eh[ec // 8][:, ec % 8, :],
                                start=(ec == 0), stop=(ec == ECH - 1))

                    VLEAD = 3
                    for ec in range(VLEAD):
                        emit_v(ec)
                    for ec in range(ECH):
                        if ec % 8 == 0 and ec // 8 + 1 < 4:
                            load_group(ec // 8 + 1)
                            if tb == 0:
                                load_wqk0(ec // 8 + 1)
                        emit_qk(ec)
                        if ec + VLEAD < ECH:
                            emit_v(ec + VLEAD)
                    drain_qk(p1st, pst, 0, tb)
                    for tc_i in range(2):
                        eng = nc.scalar if tc_i == 0 else nc.vector
                        _copy(nc, eng, v_all[:, tb * 2 + tc_i, :],
                              ps_v[tc_i][:])

            # ====== phase 1b: qk(h1) || attn(b0), then attn(b1 lo) + rs ======
            import contextlib
            state = {"consumed": set(), "done": set()}

            def run_iters(pools, padded, proj_fn, lo, hi, do_chunks,
                          prefetch=True):
                for i in range(lo, hi):
                    if proj_fn is not None:
                        proj_fn(i)
                    s = padded[i] if i < len(padded) else None
                    if s is not None:
                        produce(pools, *s)
                    if prefetch:
                        nxt = padded[i + 2] if i + 2 < len(padded) else None
                        if nxt is not None:
                            prefetch_kp(pools, *nxt)
                    nq = padded[i + 1] if i + 1 < len(padded) else None
                    if nq is not None:
                        prefetch_qs(pools, *nq)
                    ps = padded[i - 3] if 3 <= i < len(padded) + 3 else None
                    if ps is not None:
                        consume(pools, *ps)
                        state["consumed"].add(ps)
                        if do_chunks:
                            for ci in range(len(CHUNKS)):
                                if ci not in state["done"] and \
                                        chunk_reqs(ci) <= state["consumed"]:
                                    state["done"].add(ci)
                                    proj_rs(pools, ci)

            with (
                nc.named_scope("proj_b_attn0"),
                contextlib.ExitStack() as stack,
            ):
                pools = make_attn_pools(stack, with_proj_out=False)
                pools["dma_q"] = nc.gpsimd
                pools["tq"] = nc.scalar

                padded = [(0, 0, u) for u in range(SUP)] + \
                         [(0, 1, u) for u in range(SUP)] + \
                         [None, None] + \
                         [(1, 0, u) for u in range(4)] + \
                         [(1, 1, u) for u in range(4)]

                with (
                    tc.tile_pool(name="p2w", bufs=1) as p2w,
                    tc.tile_pool(name="p2hs", bufs=2) as p2hs,
                    tc.tile_pool(name="p2st", bufs=2) as p2st,
                    tc.tile_pool(name="psB", bufs=1, space="PSUM") as psB,
                ):
                    wqk1_sb = p2w.tile([128, ECH, 512], F32R, name="wqk1_sb")

                    def load_wqk1(ci):
                        e0 = ci * (ECH // 4)
                        e1 = (ci + 1) * (ECH // 4)
                        nc.sync.dma_start(
                            wqk1_sb[:, e0:e1, :],
                            wqk.ap()[e0:e1, :, 512:1024]
                            .rearrange("ec p f -> p ec f"))

                    load_wqk1(0)

                    def proj_h1(tb):
                        hs_eh = {}

                        def load_group(g):
                            hs_t = p2hs.tile([128, 8, 256], F32R,
                                             name="hs2_t", tag="hs2_t")
                            nc.sync.dma_start(
                                hs_t[:],
                                hs5.ap()[tb, g * 8:(g + 1) * 8]
                                .rearrange("ec p t -> p ec t"))
                            hs_eh[g] = hs_t

                        load_group(0)
                        pst = [psB.tile([128, 256], F32, name=f"ps_qk1{fc}",
                                        tag=f"ps_qk1{fc}") for fc in range(4)]
                        for ec in range(ECH):
                            if ec % 8 == 0 and ec // 8 + 1 < 4:
                                load_group(ec // 8 + 1)
                                if tb == 0:
                                    load_wqk1(ec // 8 + 1)
                            for fc in range(4):
                                nc.tensor.matmul(
                                    pst[fc][:],
                                    wqk1_sb[:, ec, fc * 128:(fc + 1) * 128],
                                    hs_eh[ec // 8][:, ec % 8, :],
                                    start=(ec == 0), stop=(ec == ECH - 1))
                        drain_qk(p2st, pst, 1, tb)

                    def proj_h1_delayed(i):
                        if 2 <= i < NTB + 2:
                            proj_h1(i - 2)

                    for k in range(2):
                        prefetch_kp(pools, *padded[k])
                    prefetch_qs(pools, *padded[0])
                    run_iters(pools, padded, proj_h1_delayed, 0, NTB + 2,
                              do_chunks=False)

                # proj pools closed: b1 low supers + b0/b1-lo outproj + RS
                with contextlib.ExitStack() as stack2:
                    p2 = dict(pools)
                    add_proj_out_pools(stack2, p2)
                    p2["dma_q"] = nc.sync
                    run_iters(p2, padded, None, NTB + 2, len(padded) + 3,
                              do_chunks=True)
                kp_tiles.clear()

            # ============ phase 2b: attn(b1 hi) + proj + rs ============
            with (
                nc.named_scope("attn1"),
                contextlib.ExitStack() as stack,
            ):
                pools = make_attn_pools(stack, with_proj_out=True)
                pools["dma_q"] = nc.sync
                pools["tq"] = nc.sync
                pools["kp_bufs"] = 2
                stages_b1 = [(1, h, u) for u in range(4, SUP)
                             for h in range(2)]
                seen = set()
                for (b, h, u) in stages_b1:
                    for j in range(u + 1):
                        if (b, h, j) not in seen:
                            seen.add((b, h, j))
                            prefetch_kp(pools, b, h, j)
                prefetch_qs(pools, *stages_b1[0])
                run_iters(pools, stages_b1, None, 0, len(stages_b1) + 3,
                          do_chunks=True, prefetch=False)
                kp_tiles.clear()

    nc.finalize()
    return nc


def prep_inputs(cfg: Cfg, hidden_states, w_qkv, w_proj):
    """Shard + lay out the full inputs for each of the 8 cores."""
    import ml_dtypes
    seq, batch, e = hidden_states.shape
    assert (seq, batch, e) == (cfg.seq, cfg.batch, cfg.e)
    hs_t = np.ascontiguousarray(
        hidden_states.transpose(1, 0, 2).reshape(cfg.tok, e).T
    )  # [e, tok], tokens batch-major
    hs5 = np.ascontiguousarray(
        hs_t.reshape(cfg.ech, 128, cfg.ntb, 256).transpose(2, 0, 1, 3)
    ).astype(np.float32)

    scale = math.sqrt(math.sqrt(KV_CHANNELS))
    w3 = w_qkv.reshape(HEADS, 3, HD, e)
    mask = np.full((128, 1024), 0.0, dtype=np.float32)
    cols = np.arange(1024)[None, :]
    rows = np.arange(128)[:, None]
    mask[cols > 384 + rows] = NEG
    mask = np.ascontiguousarray(mask[:, 256:640])  # only cols 256..640 used
    ident = np.eye(128, dtype=ml_dtypes.bfloat16)

    in_maps = []
    for c in range(N_CORES):
        hsel = [2 * c, 2 * c + 1]
        # head-major qk: [h, (q(256), k(256))] -> [1024, e]
        wqk_rows = []
        for h in hsel:
            wqk_rows.append((w3[h, 0] * scale).reshape(HD, e))
            wqk_rows.append((w3[h, 1] * scale).reshape(HD, e))
        wqk_cat = np.concatenate(wqk_rows, axis=0)  # [1024, e] (q0,k0,q1,k1)
        # reorder to h*512 + (q,k): currently [q0(256),k0,q1,k1] == desired
        wqk_t = np.ascontiguousarray(wqk_cat.T.reshape(cfg.ech, 128, 1024))
        w_v = np.concatenate([w3[h, 2].reshape(HD, e) for h in hsel], axis=0)
        wv_t = np.ascontiguousarray(w_v.T.reshape(cfg.ech, 128, 512))
        wp_c = w_proj[:, c * 512:(c + 1) * 512]  # [OUT, 512]
        wp_t = np.ascontiguousarray(wp_c.T.reshape(4, 128, cfg.out))
        in_maps.append({
            "hs5": hs5,
            "wqk": wqk_t.astype(np.float32),
            "wv": wv_t.astype(ml_dtypes.bfloat16),
            "wp": wp_t.astype(ml_dtypes.bfloat16),
            "maskm": mask,
            "identm": ident,
        })
    return in_maps


def assemble_output(cfg: Cfg, results):
    """Gather per-core ReduceScatter shards into the full [seq, b, out]."""
    full = np.empty((cfg.tok, cfg.out), dtype=np.float32)
    for (t0, nt, _b, _parts) in CHUNKS:
        rows = nt // N_CORES
        for r in range(N_CORES):
            shard = results[r]["out_ext"][t0 // N_CORES:
                                          t0 // N_CORES + rows]
            full[t0 + r * rows:t0 + (r + 1) * rows] = \
                shard.astype(np.float32)
    return np.ascontiguousarray(
        full.reshape(cfg.batch, cfg.seq, cfg.out).transpose(1, 0, 2))


_NC_CACHE = {}


def run(cfg: Cfg, hidden_states, w_qkv, w_proj, trace=False):
    key = (cfg.seq, cfg.e, cfg.out)
    if key not in _NC_CACHE:
        _NC_CACHE[key] = build_kernel(cfg)
    nc = _NC_CACHE[key]
    in_maps = prep_inputs(cfg, hidden_states, w_qkv, w_proj)
    res = bass_utils.run_bass_kernel_spmd(
        nc, in_maps, core_ids=list(range(N_CORES)), trace=trace)
    return assemble_output(cfg, res.results), res


def kernel(hidden_states, attention_mask, w_qkv, w_proj):
    cfg = Cfg()
    out, _ = run(cfg, np.asarray(hidden_states, dtype=np.float32),
                 np.asarray(w_qkv, dtype=np.float32),
                 np.asarray(w_proj, dtype=np.float32))
    return out


# revision 30
# speedup vs baseline: 1.1480x; 1.0070x over previous
"""Trainium2 Bass kernel for nn_CausalSelfAttention_78331613544603.

Tensor-parallel over heads across 8 NeuronCores (Megatron-style).
Per core (2 heads), three software-pipelined phases:
  1a: QK projection for head0 + V projection for both heads (bf16).
  1b: QK projection for head1 on the PE, interleaved with causal
      attention for batch0 (both heads), batch0 output projection and
      its chunked ReduceScatter.
  2b: attention for batch1 + output projection + ReduceScatter.
Scores run in fp32r (1 cycle/row); P^T is produced by DMA transpose
(xbar) instead of PE transposes; partial sums reduce in bf16.

Self-contained: only needs numpy + the concourse toolchain staged at
/opt/trn_rl_repo (also importable via the environment's PYTHONPATH).
"""

import math
import sys

import numpy as np

try:
    import concourse.bass as bass
except ImportError:
    sys.path.insert(0, "/opt/trn_rl_repo")
    import concourse.bass as bass

import concourse.mybir as mybir
import concourse.tile as tile
from concourse import bacc, bass_utils

F32 = mybir.dt.float32
F32R = mybir.dt.float32r
BF16 = mybir.dt.bfloat16
FP16 = mybir.dt.float16
FP8 = mybir.dt.float8e4
LOSC = 32.0

N_CORES = 8
HEADS = 16
HPC = HEADS // N_CORES  # heads per core = 2
HD = 256  # head dim
KV_CHANNELS = 128
NEG = -1.0e30


def _copy(nc, eng, dst, src):
    if eng is nc.scalar:
        nc.scalar.copy(dst, src)
    else:
        eng.tensor_copy(dst, src)


# RS chunks: (token_start, n_tokens, batch, [(sl, st0, nst), ...]).
# b0 chunks run as phase-2b fillers; b1 chunks fire as their supers
# complete (2b processes supers in descending order, so high sls first;
# the last two chunks are 256 tokens to shrink the collective tail).
CHUNKS = [
    (0, 1024, 0, [(0, 0, 4), (1, 0, 4)]),
    (1024, 1024, 0, [(2, 0, 4), (3, 0, 4)]),
    (2048, 512, 1, [(0, 0, 4)]),
    (2560, 512, 1, [(1, 0, 4)]),
    (3072, 512, 1, [(2, 0, 4)]),
    (3584, 256, 1, [(3, 0, 2)]),
    (3840, 256, 1, [(3, 2, 2)]),
]


def chunk_reqs(ci):
    t0, nt, b, parts = CHUNKS[ci]
    req = set()
    for (sl, st0, nst) in parts:
        for st in range(st0, st0 + nst):
            u = 2 * sl + (1 if st >= 2 else 0)
            for h in range(2):
                req.add((b, h, u))
    return req


class Cfg:
    def __init__(self, seq=2048, e=4096, out=2048):
        self.seq = seq
        self.batch = 2
        self.e = e  # input embedding dim (2*HIDDEN)
        self.out = out  # output dim (HIDDEN)
        self.ech = e // 128  # contraction chunks (32)
        self.tok = seq * self.batch  # total tokens, batch-major (4096)
        self.ntb = self.tok // 256  # projection token blocks (16)
        self.supers = seq // 256  # q super-tiles per (b,h) (8)
        self.nstg = self.tok // 512  # reduce-scatter chunks (8)


def build_kernel(cfg: Cfg):
    nc = bacc.Bacc("TRN2", target_bir_lowering=False, debug=False,
                   num_devices=N_CORES)

    ECH = cfg.ech  # 32
    SEQ = cfg.seq
    TOK = cfg.tok
    OUT = cfg.out
    NTB = cfg.ntb
    SUP = cfg.supers

    # ---- kernel I/O ----
    hs5 = nc.dram_tensor("hs5", [NTB, ECH, 128, 256], F32R,
                         kind="ExternalInput")
    wqk = nc.dram_tensor("wqk", [ECH, 128, 1024], F32R,
                         kind="ExternalInput")  # head-major: h*512+(q,k)
    wv = nc.dram_tensor("wv", [ECH, 128, 512], BF16, kind="ExternalInput")
    wp = nc.dram_tensor("wp", [4, 128, OUT], BF16, kind="ExternalInput")
    maskm = nc.dram_tensor("maskm", [128, 384], F32, kind="ExternalInput")
    identm = nc.dram_tensor("identm", [128, 128], BF16, kind="ExternalInput")
    out_ext = nc.dram_tensor("out_ext", [cfg.tok // N_CORES, OUT],
                             BF16, kind="ExternalOutput")

    with tile.TileContext(nc) as tc:
        with (
            tc.tile_pool(name="const", bufs=1) as constp,
            tc.tile_pool(name="resident", bufs=1) as resp,
            tc.tile_pool(name="dram", bufs=1, space="DRAM") as dramp,
        ):
            mask_sb = constp.tile([128, 384], F32, name="mask_sb")
            nc.sync.dma_start(mask_sb[:], maskm.ap())
            ident_holder = {}

            # v for all tokens / both heads, resident through attention
            v_all = resp.tile([128, TOK // 128, 512], BF16, name="v_all")
            # y^T accumulators: 4 slots, reused across batches
            yt_t = {}

            def get_yt(b, sl):
                if (b, sl) not in yt_t:
                    yt_t[(b, sl)] = resp.tile(
                        [128, 4, 512], BF16, name=f"yt_{b}_{sl}",
                        tag=f"yt_{sl}")
                return yt_t[(b, sl)]

            qh_spill = dramp.tile([2, 128, NTB, 2, 256], FP16,
                                  name="qh_spill")
            kh_spill = dramp.tile([2, 128, NTB, 2, 256], FP16,
                                  name="kh_spill")
            q8_spill = dramp.tile([2, 128, NTB, 2, 2, 256], FP8,
                                  name="q8_spill")
            k8_spill = dramp.tile([2, 128, NTB, 2, 2, 256], FP8,
                                  name="k8_spill")
            partial_c = [dramp.tile([nt, OUT], BF16, name=f"partial{ci}",
                                    tag=f"partial{ci}")
                         for ci, (t0, nt, b, parts) in enumerate(CHUNKS)]
            rs_out_c = [dramp.tile([nt // N_CORES, OUT], BF16,
                                   name=f"rs_out{ci}", tag=f"rs_out{ci}")
                        for ci, (t0, nt, b, parts) in enumerate(CHUNKS)]

            # ====== shared attention machinery (used by 1b and 2b) ======
            def make_attn_pools(stack, with_proj_out):
                pools = {}
                pools["kp"] = stack.enter_context(
                    tc.tile_pool(name="kp", bufs=1))
                pools["qs"] = stack.enter_context(
                    tc.tile_pool(name="qs", bufs=3))
                pools["strip"] = stack.enter_context(
                    tc.tile_pool(name="strip", bufs=2))
                pools["pstrip"] = stack.enter_context(
                    tc.tile_pool(name="pstrip", bufs=2))
                pools["pt"] = stack.enter_context(
                    tc.tile_pool(name="pt", bufs=2))
                pools["stat"] = stack.enter_context(
                    tc.tile_pool(name="stat", bufs=2))
                pools["ps_s"] = stack.enter_context(
                    tc.tile_pool(name="ps_s", bufs=2, space="PSUM"))
                pools["ps_y"] = stack.enter_context(
                    tc.tile_pool(name="ps_y", bufs=1, space="PSUM"))
                if with_proj_out:
                    add_proj_out_pools(stack, pools)
                return pools

            def add_proj_out_pools(stack, pools):
                pools["ps_pt"] = stack.enter_context(
                    tc.tile_pool(name="ps_pt", bufs=2, space="PSUM"))
                pools["wpp"] = stack.enter_context(
                    tc.tile_pool(name="wpp", bufs=1))
                ident_sb = pools["wpp"].tile([128, 128], BF16,
                                             name="ident_sb")
                nc.sync.dma_start(ident_sb[:], identm.ap())
                ident_holder["t"] = ident_sb
                pools["ost"] = stack.enter_context(
                    tc.tile_pool(name="ost", bufs=2))
                pools["ps_o"] = stack.enter_context(
                    tc.tile_pool(name="ps_o", bufs=2, space="PSUM"))
                wp_sb = pools["wpp"].tile([128, 4, OUT], BF16, name="wp_sb")
                for ci in range(2):
                    nc.scalar.dma_start(
                        wp_sb[:, 2 * ci:2 * ci + 2, :],
                        wp.ap()[2 * ci:2 * ci + 2]
                        .rearrange("fc p o -> p fc o"))
                pools["wp_sb"] = wp_sb

            kp_tiles = {}
            qs_tiles = {}
            pt_tiles = {}

            def prefetch_kp(pools, b, h, u):
                """Load K increment j=u: fp16 hi + fp8 cross pieces."""
                nb = 2 if pools.get("kp_bufs", 1) > 1 else 1
                kp_t = pools["kp"].tile([128, 2, 256], FP16, name="kph",
                                        tag=f"kph{u}", bufs=nb)
                pools["dma_q"].dma_start(kp_t[:], kh_spill[h, :, b * 8 + u])
                kp8_t = pools["kp"].tile([128, 2, 2, 256], FP8, name="kp8",
                                         tag=f"kp8{u}", bufs=nb)
                pools["dma_q"].dma_start(kp8_t[:], k8_spill[h, :, b * 8 + u])
                kp_tiles[(b, h, u)] = (kp_t, kp8_t)

            def prefetch_qs(pools, b, h, u):
                qs_t = pools["qs"].tile([128, 2, 256], FP16, name="qsh",
                                        tag="qsh", bufs=2)
                pools["dma_q"].dma_start(qs_t[:], qh_spill[h, :, b * 8 + u])
                qs8_t = pools["qs"].tile([128, 2, 2, 256], FP8, name="qs8",
                                         tag="qs8", bufs=2)
                pools["dma_q"].dma_start(qs8_t[:], q8_spill[h, :, b * 8 + u])
                qs_tiles[(b, h, u)] = (qs_t, qs8_t)

            def produce(pools, b, h, u):
                """scores for super u -> softmax -> P^T via DMA transpose."""
                qs_t, qs8_t = qs_tiles.pop((b, h, u))
                nk = 2 * (u + 1)
                Lp = nk * 128
                pt_sb = pools["pt"].tile([128, 2 * (SEQ // 128), 128], BF16,
                                         name="pt", tag="pt")
                pt_tiles[(b, h, u)] = pt_sb
                pstrip2 = pools["pstrip"].tile([128, 2 * SEQ], BF16,
                                               name="pstrip", tag="pstrip")
                for qt in range(2):
                    i = 2 * u + qt
                    strip = pools["strip"].tile([128, SEQ], F32,
                                                name="strip", tag="strip")
                    for j in range(u + 1):
                        ps_s = pools["ps_s"].tile([128, 256], F32,
                                                  name="ps_s", tag="ps_s")
                        kp_t, kp8_t = kp_tiles[(b, h, j)]
                        for ec in range(2):
                            nc.tensor.matmul(
                                ps_s[:],
                                qs_t[:, ec, qt * 128:(qt + 1) * 128],
                                kp_t[:, ec, :],
                                start=(ec == 0), stop=False)
                        for ec in range(2):
                            nc.tensor.matmul(
                                ps_s[:],
                                qs8_t[:, ec, :, qt * 128:(qt + 1) * 128],
                                kp8_t[:, ec, :, :],
                                perf_mode=mybir.MatmulPerfMode.DoubleRow,
                                start=False, stop=(ec == 1))
                        dst = strip[:, j * 256:(j + 1) * 256]
                        if j == u:
                            off = 128 if qt == 0 else 0
                            nc.vector.tensor_tensor(
                                dst, ps_s[:], mask_sb[:, off:off + 256],
                                mybir.AluOpType.add)
                        elif j % 2 == 0:
                            nc.scalar.copy(dst, ps_s[:])
                        else:
                            nc.vector.tensor_copy(dst, ps_s[:])
                    Lv = (i + 1) * 128
                    negmax = pools["stat"].tile([128, 1], F32, name="negmax",
                                                tag="negmax")
                    nc.vector.reduce_max(negmax[:], strip[:, :Lv],
                                         axis=mybir.AxisListType.X,
                                         negate=True)
                    pstrip = pstrip2[:, qt * Lp:(qt + 1) * Lp]
                    zsum = pools["stat"].tile([128, 1], F32, name="zsum",
                                              tag="zsum")
                    nc.scalar.activation(
                        pstrip, strip[:, :Lp],
                        mybir.ActivationFunctionType.Exp,
                        bias=negmax[:], scale=1.0, accum_out=zsum[:])
                    rz = pools["stat"].tile([128, 1], F32, name="rz",
                                            tag="rz")
                    nc.vector.reciprocal(rz[:], zsum[:])
                    nc.vector.tensor_scalar_mul(pstrip, pstrip, rz[:])
                if "ps_pt" in pools:
                    for c in range(nk):
                        ps_pt = pools["ps_pt"].tile([128, 256], BF16,
                                                    name="ps_pt", tag="ps_pt")
                        for qt in range(2):
                            nc.tensor.transpose(
                                ps_pt[:, qt * 128:(qt + 1) * 128],
                                pstrip2[:, qt * Lp + c * 128:
                                        qt * Lp + (c + 1) * 128],
                                ident_holder["t"][:])
                        for qt in range(2):
                            eng = nc.vector if (c + qt) % 2 == 0 \
                                else nc.scalar
                            _copy(nc, eng, pt_sb[:, qt * nk + c, :],
                                  ps_pt[:, qt * 128:(qt + 1) * 128])
                else:
                    pools["tq"].dma_start_transpose(
                        pt_sb[:, :2 * nk, :], pstrip2[:, :2 * Lp])

            def consume(pools, b, h, u):
                """PV for super u -> y^T chunk."""
                nk = 2 * (u + 1)
                pt_sb = pt_tiles.pop((b, h, u))
                ps_y = [pools["ps_y"].tile([128, 256], F32, name=f"ps_y{dh}",
                                           tag=f"ps_y{dh}")
                        for dh in range(2)]
                for c in range(nk):
                    g = b * (SEQ // 128) + c
                    for dh in range(2):
                        nc.tensor.matmul(
                            ps_y[dh][:],
                            v_all[:, g, h * 256 + dh * 128:
                                  h * 256 + (dh + 1) * 128],
                            pt_sb[:, c:c + nk + 1:nk, :],
                            start=(c == 0), stop=(c == nk - 1))
                yt = get_yt(b, u // 2)
                for dh in range(2):
                    eng = nc.vector if dh == 0 else nc.scalar
                    _copy(nc, eng,
                          yt[:, 2 * h + dh, (u % 2) * 256:(u % 2 + 1) * 256],
                          ps_y[dh][:])

            def proj_rs(pools, ci):
                """output projection for one RS chunk."""
                t0_tok, nt, b, parts = CHUNKS[ci]
                wp_sb = pools["wp_sb"]
                row = 0
                for (sl, st0, nst) in parts:
                    yt = yt_t[(b, sl)]
                    for st in range(st0, st0 + nst):
                        ost = pools["ost"].tile([128, OUT], BF16, name="ost",
                                                tag="ost")
                        for ob in range(OUT // 512):
                            ps_o = pools["ps_o"].tile([128, 512], F32,
                                                      name="ps_o", tag="ps_o")
                            for fc in range(4):
                                nc.tensor.matmul(
                                    ps_o[:],
                                    yt[:, fc, st * 128:(st + 1) * 128],
                                    wp_sb[:, fc, ob * 512:(ob + 1) * 512],
                                    start=(fc == 0), stop=(fc == 3))
                            eng = nc.vector if ob % 2 == 0 else nc.scalar
                            _copy(nc, eng, ost[:, ob * 512:(ob + 1) * 512],
                                  ps_o[:])
                        nc.sync.dma_start(partial_c[ci][row:row + 128, :],
                                          ost[:])
                        row += 128
                nc.gpsimd.collective_compute(
                    "ReduceScatter",
                    mybir.AluOpType.add,
                    ins=[partial_c[ci].opt()],
                    outs=[rs_out_c[ci].opt()],
                    replica_groups=[list(range(N_CORES))],
                )
                nc.gpsimd.dma_start(
                    out_ext.ap()[t0_tok // N_CORES:
                                 (t0_tok + nt) // N_CORES],
                    rs_out_c[ci])

            def drain_qk(stpool, pst, hidx, tb):
                """Drain 4 qk PSUM groups into fp16-hi + scaled-fp8 spills.
                q pieces: (lo*32, hi/32); k pieces: (hi/32, lo*32)."""
                for pair in range(2):
                    hi = stpool.tile([128, 2, 256], FP16, name="hi",
                                     tag="hi")
                    p8 = stpool.tile([128, 2, 2, 256], FP8, name="p8",
                                     tag="p8")
                    for half in range(2):
                        fc = pair * 2 + half
                        eng = nc.vector if half == 0 else nc.scalar
                        _copy(nc, eng, hi[:, half, :], pst[fc][:])
                        lo = stpool.tile([128, 256], FP16, name="lo",
                                         tag="lo")
                        nc.vector.tensor_tensor(lo[:], pst[fc][:],
                                                hi[:, half, :],
                                                mybir.AluOpType.subtract)
                        lo_pc = 0 if pair == 0 else 1
                        nc.gpsimd.tensor_scalar_mul(
                            p8[:, half, lo_pc, :], lo[:], LOSC)
                        nc.gpsimd.tensor_scalar_mul(
                            p8[:, half, 1 - lo_pc, :], hi[:, half, :],
                            1.0 / LOSC)
                    hdst = qh_spill if pair == 0 else kh_spill
                    dst8 = q8_spill if pair == 0 else k8_spill
                    nc.sync.dma_start(hdst[hidx, :, tb], hi[:])
                    nc.sync.dma_start(dst8[hidx, :, tb], p8[:])

            # ================= phase 1a: qk(h0) + v(both) =================
            with (
                nc.named_scope("proj_a"),
                tc.tile_pool(name="p1w", bufs=1) as p1w,
                tc.tile_pool(name="p1hs", bufs=3) as p1hs,
                tc.tile_pool(name="p1hb", bufs=3) as p1hb,
                tc.tile_pool(name="p1st", bufs=6) as p1st,
                tc.tile_pool(name="psA", bufs=1, space="PSUM") as psA,
                tc.tile_pool(name="psV", bufs=2, space="PSUM") as psV,
            ):
                wqk_sb = p1w.tile([128, ECH, 512], F32R, name="wqk0_sb")
                wv_sb = p1w.tile([128, ECH, 512], BF16, name="wv_sb")

                def load_wqk0(ci):
                    e0 = ci * (ECH // 4)
                    e1 = (ci + 1) * (ECH // 4)
                    nc.sync.dma_start(
                        wqk_sb[:, e0:e1, :],
                        wqk.ap()[e0:e1, :, 0:512].rearrange("ec p f -> p ec f"))
                    nc.scalar.dma_start(
                        wv_sb[:, e0:e1, :],
                        wv.ap()[e0:e1].rearrange("ec p f -> p ec f"))


                load_wqk0(0)
                for tb in range(NTB):
                    hs_eh = {}
                    hb_eh = {}

                    def load_group(g, tb=tb, hs_eh=hs_eh, hb_eh=hb_eh):
                        hs_t = p1hs.tile([128, 8, 256], F32R, name="hs_t",
                                         tag="hs_t")
                        nc.sync.dma_start(
                            hs_t[:],
                            hs5.ap()[tb, g * 8:(g + 1) * 8]
                            .rearrange("ec p t -> p ec t"))
                        hb_t = p1hb.tile([128, 8, 256], BF16, name="hb_t",
                                         tag="hb_t")
                        nc.gpsimd.tensor_copy(hb_t[:], hs_t[:])
                        hs_eh[g] = hs_t
                        hb_eh[g] = hb_t

                    load_group(0)
                    pst = [psA.tile([128, 256], F32, name=f"ps_qk{fc}",
                                    tag=f"ps_qk{fc}") for fc in range(4)]
                    ps_v = [psV.tile([128, 512], F32, name=f"ps_v{tc_i}",
                                     tag=f"ps_v{tc_i}") for tc_i in range(2)]

                    def emit_v(ec):
                        for tc_i in range(2):
                            nc.tensor.matmul(
                                ps_v[tc_i][:],
                                hb_eh[ec // 8][:, ec % 8,
                                               tc_i * 128:(tc_i + 1) * 128],
                                wv_sb[:, ec, :],
                                start=(ec == 0), stop=(ec == ECH - 1))

                    def emit_qk(ec):
                        for fc in range(4):
                            nc.tensor.matmul(
                                pst[fc][:],
                                wqk_sb[:, ec, fc * 128:(fc + 1) * 128],
                                hs_eh[ec // 8][:, ec % 8, :],
                                start=(ec == 0), stop=(ec == ECH - 1))

                    VLEAD = 3
                    for ec in range(VLEAD):
                        emit_v(ec)
                    for ec in range(ECH):
                        if ec % 8 == 0 and ec // 8 + 1 < 4:
                            load_group(ec // 8 + 1)
                            if tb == 0:
                                load_wqk0(ec // 8 + 1)
                        emit_qk(ec)
                        if ec + VLEAD < ECH:
                            emit_v(ec + VLEAD)
                    drain_qk(p1st, pst, 0, tb)
                    for tc_i in range(2):
                        eng = nc.scalar if tc_i == 0 else nc.vector
                        _copy(nc, eng, v_all[:, tb * 2 + tc_i, :],
                              ps_v[tc_i][:])

            # ====== phase 1b: qk(h1) || attn(b0), then attn(b1 lo) + rs ======
            import contextlib
            state = {"consumed": set(), "done": set()}

            def run_iters(pools, padded, proj_fn, lo, hi, do_chunks,
                          prefetch=True):
                for i in range(lo, hi):
                    if proj_fn is not None:
                        proj_fn(i)
                    s = padded[i] if i < len(padded) else None
                    if s is not None:
                        produce(pools, *s)
                    if prefetch:
                        nxt = padded[i + 2] if i + 2 < len(padded) else None
                        if nxt is not None:
                            prefetch_kp(pools, *nxt)
                    nq = padded[i + 1] if i + 1 < len(padded) else None
                    if nq is not None:
                        prefetch_qs(pools, *nq)
                    ps = padded[i - 3] if 3 <= i < len(padded) + 3 else None
                    if ps is not None:
                        consume(pools, *ps)
                        state["consumed"].add(ps)
                        if do_chunks:
                            for ci in range(len(CHUNKS)):
                                if ci not in state["done"] and \
                                        chunk_reqs(ci) <= state["consumed"]:
                                    state["done"].add(ci)
                                    proj_rs(pools, ci)

            with (
                nc.named_scope("proj_b_attn0"),
                contextlib.ExitStack() as stack,
            ):
                pools = make_attn_pools(stack, with_proj_out=False)
                pools["dma_q"] = nc.gpsimd
                pools["tq"] = nc.scalar

                padded = [(0, 0, u) for u in range(SUP)] + \
                         [(0, 1, u) for u in range(SUP)] + \
                         [None, None] + \
                         [(1, 0, u) for u in range(4)] + \
                         [(1, 1, u) for u in range(4)]

                with (
                    tc.tile_pool(name="p2w", bufs=1) as p2w,
                    tc.tile_pool(name="p2hs", bufs=2) as p2hs,
                    tc.tile_pool(name="p2st", bufs=2) as p2st,
                    tc.tile_pool(name="psB", bufs=1, space="PSUM") as psB,
                ):
                    wqk1_sb = p2w.tile([128, ECH, 512], F32R, name="wqk1_sb")

                    def load_wqk1(ci):
                        e0 = ci * (ECH // 4)
                        e1 = (ci + 1) * (ECH // 4)
                        nc.sync.dma_start(
                            wqk1_sb[:, e0:e1, :],
                            wqk.ap()[e0:e1, :, 512:1024]
                            .rearrange("ec p f -> p ec f"))

                    load_wqk1(0)

                    def proj_h1(tb):
                        hs_eh = {}

                        def load_group(g):
                            hs_t = p2hs.tile([128, 8, 256], F32R,
                                             name="hs2_t", tag="hs2_t")
                            nc.sync.dma_start(
                                hs_t[:],
                                hs5.ap()[tb, g * 8:(g + 1) * 8]
                                .rearrange("ec p t -> p ec t"))
                            hs_eh[g] = hs_t

                        load_group(0)
                        pst = [psB.tile([128, 256], F32, name=f"ps_qk1{fc}",
                                        tag=f"ps_qk1{fc}") for fc in range(4)]
                        for ec in range(ECH):
                            if ec % 8 == 0 and ec // 8 + 1 < 4:
                                load_group(ec // 8 + 1)
                                if tb == 0:
                                    load_wqk1(ec // 8 + 1)
                            for fc in range(4):
                                nc.tensor.matmul(
                                    pst[fc][:],
                                    wqk1_sb[:, ec, fc * 128:(fc + 1) * 128],
                                    hs_eh[ec // 8][:, ec % 8, :],
                                    start=(ec == 0), stop=(ec == ECH - 1))
                        drain_qk(p2st, pst, 1, tb)

                    def proj_h1_delayed(i):
                        if 2 <= i < NTB + 2:
                            proj_h1(i - 2)

                    for k in range(2):
                        prefetch_kp(pools, *padded[k])
                    prefetch_qs(pools, *padded[0])
                    run_iters(pools, padded, proj_h1_delayed, 0, NTB + 2,
                              do_chunks=False)

                # proj pools closed: b1 low supers + b0/b1-lo outproj + RS
                with contextlib.ExitStack() as stack2:
                    p2 = dict(pools)
                    add_proj_out_pools(stack2, p2)
                    p2["dma_q"] = nc.sync
                    run_iters(p2, padded, None, NTB + 2, len(padded) + 3,
                              do_chunks=True)
                kp_tiles.clear()

            # ============ phase 2b: attn(b1 hi) + proj + rs ============
            with (
                nc.named_scope("attn1"),
                contextlib.ExitStack() as stack,
            ):
                pools = make_attn_pools(stack, with_proj_out=True)
                pools["dma_q"] = nc.sync
                pools["tq"] = nc.sync
                pools["kp_bufs"] = 2
                stages_b1 = [(1, h, u) for u in (4, 6, 5, 7)
                             for h in range(2)]
                seen = set()
                for (b, h, u) in stages_b1:
                    for j in range(u + 1):
                        if (b, h, j) not in seen:
                            seen.add((b, h, j))
                            prefetch_kp(pools, b, h, j)
                prefetch_qs(pools, *stages_b1[0])
                run_iters(pools, stages_b1, None, 0, len(stages_b1) + 3,
                          do_chunks=True, prefetch=False)
                kp_tiles.clear()

    nc.finalize()
    return nc


def prep_inputs(cfg: Cfg, hidden_states, w_qkv, w_proj):
    """Shard + lay out the full inputs for each of the 8 cores."""
    import ml_dtypes
    seq, batch, e = hidden_states.shape
    assert (seq, batch, e) == (cfg.seq, cfg.batch, cfg.e)
    hs_t = np.ascontiguousarray(
        hidden_states.transpose(1, 0, 2).reshape(cfg.tok, e).T
    )  # [e, tok], tokens batch-major
    hs5 = np.ascontiguousarray(
        hs_t.reshape(cfg.ech, 128, cfg.ntb, 256).transpose(2, 0, 1, 3)
    ).astype(np.float32)

    scale = math.sqrt(math.sqrt(KV_CHANNELS))
    w3 = w_qkv.reshape(HEADS, 3, HD, e)
    mask = np.full((128, 1024), 0.0, dtype=np.float32)
    cols = np.arange(1024)[None, :]
    rows = np.arange(128)[:, None]
    mask[cols > 384 + rows] = NEG
    mask = np.ascontiguousarray(mask[:, 256:640])  # only cols 256..640 used
    ident = np.eye(128, dtype=ml_dtypes.bfloat16)

    in_maps = []
    for c in range(N_CORES):
        hsel = [2 * c, 2 * c + 1]
        # head-major qk: [h, (q(256), k(256))] -> [1024, e]
        wqk_rows = []
        for h in hsel:
            wqk_rows.append((w3[h, 0] * scale).reshape(HD, e))
            wqk_rows.append((w3[h, 1] * scale).reshape(HD, e))
        wqk_cat = np.concatenate(wqk_rows, axis=0)  # [1024, e] (q0,k0,q1,k1)
        # reorder to h*512 + (q,k): currently [q0(256),k0,q1,k1] == desired
        wqk_t = np.ascontiguousarray(wqk_cat.T.reshape(cfg.ech, 128, 1024))
        w_v = np.concatenate([w3[h, 2].reshape(HD, e) for h in hsel], axis=0)
        wv_t = np.ascontiguousarray(w_v.T.reshape(cfg.ech, 128, 512))
        wp_c = w_proj[:, c * 512:(c + 1) * 512]  # [OUT, 512]
        wp_t = np.ascontiguousarray(wp_c.T.reshape(4, 128, cfg.out))
        in_maps.append({
            "hs5": hs5,
            "wqk": wqk_t.astype(np.float32),
            "wv": wv_t.astype(ml_dtypes.bfloat16),
            "wp": wp_t.astype(ml_dtypes.bfloat16),
            "maskm": mask,
            "identm": ident,
        })
    return in_maps


def assemble_output(cfg: Cfg, results):
    """Gather per-core ReduceScatter shards into the full [seq, b, out]."""
    full = np.empty((cfg.tok, cfg.out), dtype=np.float32)
    for (t0, nt, _b, _parts) in CHUNKS:
        rows = nt // N_CORES
        for r in range(N_CORES):
            shard = results[r]["out_ext"][t0 // N_CORES:
                                          t0 // N_CORES + rows]
            full[t0 + r * rows:t0 + (r + 1) * rows] = \
                shard.astype(np.float32)
    return np.ascontiguousarray(
        full.reshape(cfg.batch, cfg.seq, cfg.out).transpose(1, 0, 2))


_NC_CACHE = {}


def run(cfg: Cfg, hidden_states, w_qkv, w_proj, trace=False):
    key = (cfg.seq, cfg.e, cfg.out)
    if key not in _NC_CACHE:
        _NC_CACHE[key] = build_kernel(cfg)
    nc = _NC_CACHE[key]
    in_maps = prep_inputs(cfg, hidden_states, w_qkv, w_proj)
    res = bass_utils.run_bass_kernel_spmd(
        nc, in_maps, core_ids=list(range(N_CORES)), trace=trace)
    return assemble_output(cfg, res.results), res


def kernel(hidden_states, attention_mask, w_qkv, w_proj):
    cfg = Cfg()
    out, _ = run(cfg, np.asarray(hidden_states, dtype=np.float32),
                 np.asarray(w_qkv, dtype=np.float32),
                 np.asarray(w_proj, dtype=np.float32))
    return out


# revision 33
# speedup vs baseline: 1.1637x; 1.0136x over previous
"""Trainium2 Bass kernel for nn_CausalSelfAttention_78331613544603.

Tensor-parallel over heads across 8 NeuronCores (Megatron-style).
Per core (2 heads), three software-pipelined phases:
  1a: QK projection for head0 + V projection for both heads (bf16).
  1b: QK projection for head1 on the PE, interleaved with causal
      attention for batch0 (both heads), batch0 output projection and
      its chunked ReduceScatter.
  2b: attention for batch1 + output projection + ReduceScatter.
Scores run in fp32r (1 cycle/row); P^T is produced by DMA transpose
(xbar) instead of PE transposes; partial sums reduce in bf16.

Self-contained: only needs numpy + the concourse toolchain staged at
/opt/trn_rl_repo (also importable via the environment's PYTHONPATH).
"""

import math
import sys

import numpy as np

try:
    import concourse.bass as bass
except ImportError:
    sys.path.insert(0, "/opt/trn_rl_repo")
    import concourse.bass as bass

import concourse.mybir as mybir
import concourse.tile as tile
from concourse import bacc, bass_utils

F32 = mybir.dt.float32
F32R = mybir.dt.float32r
BF16 = mybir.dt.bfloat16
FP16 = mybir.dt.float16
FP8 = mybir.dt.float8e4
LOSC = 32.0

N_CORES = 8
HEADS = 16
HPC = HEADS // N_CORES  # heads per core = 2
HD = 256  # head dim
KV_CHANNELS = 128
NEG = -1.0e30


def _copy(nc, eng, dst, src):
    if eng is nc.scalar:
        nc.scalar.copy(dst, src)
    else:
        eng.tensor_copy(dst, src)


# RS chunks: (token_start, n_tokens, batch, [(sl, st0, nst), ...]).
# b0 chunks run as phase-2b fillers; b1 chunks fire as their supers
# complete (2b processes supers in descending order, so high sls first;
# the last two chunks are 256 tokens to shrink the collective tail).
CHUNKS = [
    (0, 1024, 0, [(0, 0, 4), (1, 0, 4)]),
    (1024, 1024, 0, [(2, 0, 4), (3, 0, 4)]),
    (2048, 512, 1, [(0, 0, 4)]),
    (2560, 512, 1, [(1, 0, 4)]),
    (3072, 256, 1, [(2, 0, 2)]),
    (3328, 256, 1, [(2, 2, 2)]),
    (3584, 256, 1, [(3, 0, 2)]),
    (3840, 256, 1, [(3, 2, 2)]),
]


def chunk_reqs(ci):
    t0, nt, b, parts = CHUNKS[ci]
    req = set()
    for (sl, st0, nst) in parts:
        for st in range(st0, st0 + nst):
            u = 2 * sl + (1 if st >= 2 else 0)
            for h in range(2):
                req.add((b, h, u))
    return req


class Cfg:
    def __init__(self, seq=2048, e=4096, out=2048):
        self.seq = seq
        self.batch = 2
        self.e = e  # input embedding dim (2*HIDDEN)
        self.out = out  # output dim (HIDDEN)
        self.ech = e // 128  # contraction chunks (32)
        self.tok = seq * self.batch  # total tokens, batch-major (4096)
        self.ntb = self.tok // 256  # projection token blocks (16)
        self.supers = seq // 256  # q super-tiles per (b,h) (8)
        self.nstg = self.tok // 512  # reduce-scatter chunks (8)


def build_kernel(cfg: Cfg):
    nc = bacc.Bacc("TRN2", target_bir_lowering=False, debug=False,
                   num_devices=N_CORES)

    ECH = cfg.ech  # 32
    SEQ = cfg.seq
    TOK = cfg.tok
    OUT = cfg.out
    NTB = cfg.ntb
    SUP = cfg.supers

    # ---- kernel I/O ----
    hs5 = nc.dram_tensor("hs5", [NTB, ECH, 128, 256], F32R,
                         kind="ExternalInput")
    wqk = nc.dram_tensor("wqk", [ECH, 128, 1024], F32R,
                         kind="ExternalInput")  # head-major: h*512+(q,k)
    wv = nc.dram_tensor("wv", [ECH, 128, 512], BF16, kind="ExternalInput")
    wp = nc.dram_tensor("wp", [4, 128, OUT], BF16, kind="ExternalInput")
    maskm = nc.dram_tensor("maskm", [128, 384], F32, kind="ExternalInput")
    identm = nc.dram_tensor("identm", [128, 128], BF16, kind="ExternalInput")
    out_ext = nc.dram_tensor("out_ext", [cfg.tok // N_CORES, OUT],
                             BF16, kind="ExternalOutput")

    with tile.TileContext(nc) as tc:
        with (
            tc.tile_pool(name="const", bufs=1) as constp,
            tc.tile_pool(name="resident", bufs=1) as resp,
            tc.tile_pool(name="dram", bufs=1, space="DRAM") as dramp,
        ):
            mask_sb = constp.tile([128, 384], F32, name="mask_sb")
            nc.sync.dma_start(mask_sb[:], maskm.ap())
            ident_holder = {}

            # v for all tokens / both heads, resident through attention
            v_all = resp.tile([128, TOK // 128, 512], BF16, name="v_all")
            # y^T accumulators: 4 slots, reused across batches
            yt_t = {}

            def get_yt(b, sl):
                if (b, sl) not in yt_t:
                    yt_t[(b, sl)] = resp.tile(
                        [128, 4, 512], BF16, name=f"yt_{b}_{sl}",
                        tag=f"yt_{sl}")
                return yt_t[(b, sl)]

            qh_spill = dramp.tile([2, 128, NTB, 2, 256], FP16,
                                  name="qh_spill")
            kh_spill = dramp.tile([2, 128, NTB, 2, 256], FP16,
                                  name="kh_spill")
            q8_spill = dramp.tile([2, 128, NTB, 2, 2, 256], FP8,
                                  name="q8_spill")
            k8_spill = dramp.tile([2, 128, NTB, 2, 2, 256], FP8,
                                  name="k8_spill")
            partial_c = [dramp.tile([nt, OUT], BF16, name=f"partial{ci}",
                                    tag=f"partial{ci}")
                         for ci, (t0, nt, b, parts) in enumerate(CHUNKS)]
            rs_out_c = [dramp.tile([nt // N_CORES, OUT], BF16,
                                   name=f"rs_out{ci}", tag=f"rs_out{ci}")
                        for ci, (t0, nt, b, parts) in enumerate(CHUNKS)]

            # ====== shared attention machinery (used by 1b and 2b) ======
            def make_attn_pools(stack, with_proj_out):
                pools = {}
                pools["kp"] = stack.enter_context(
                    tc.tile_pool(name="kp", bufs=1))
                pools["qs"] = stack.enter_context(
                    tc.tile_pool(name="qs", bufs=3))
                pools["strip"] = stack.enter_context(
                    tc.tile_pool(name="strip", bufs=2))
                pools["pstrip"] = stack.enter_context(
                    tc.tile_pool(name="pstrip", bufs=2))
                pools["pt"] = stack.enter_context(
                    tc.tile_pool(name="pt", bufs=2))
                pools["stat"] = stack.enter_context(
                    tc.tile_pool(name="stat", bufs=2))
                pools["ps_s"] = stack.enter_context(
                    tc.tile_pool(name="ps_s", bufs=2, space="PSUM"))
                pools["ps_y"] = stack.enter_context(
                    tc.tile_pool(name="ps_y", bufs=1, space="PSUM"))
                if with_proj_out:
                    add_proj_out_pools(stack, pools)
                return pools

            def add_proj_out_pools(stack, pools):
                pools["ps_pt"] = stack.enter_context(
                    tc.tile_pool(name="ps_pt", bufs=2, space="PSUM"))
                pools["wpp"] = stack.enter_context(
                    tc.tile_pool(name="wpp", bufs=1))
                ident_sb = pools["wpp"].tile([128, 128], BF16,
                                             name="ident_sb")
                nc.sync.dma_start(ident_sb[:], identm.ap())
                ident_holder["t"] = ident_sb
                pools["ost"] = stack.enter_context(
                    tc.tile_pool(name="ost", bufs=2))
                pools["ps_o"] = stack.enter_context(
                    tc.tile_pool(name="ps_o", bufs=2, space="PSUM"))
                wp_sb = pools["wpp"].tile([128, 4, OUT], BF16, name="wp_sb")
                for ci in range(2):
                    nc.scalar.dma_start(
                        wp_sb[:, 2 * ci:2 * ci + 2, :],
                        wp.ap()[2 * ci:2 * ci + 2]
                        .rearrange("fc p o -> p fc o"))
                pools["wp_sb"] = wp_sb

            kp_tiles = {}
            qs_tiles = {}
            pt_tiles = {}

            def prefetch_kp(pools, b, h, u):
                """Load K increment j=u: fp16 hi + fp8 cross pieces."""
                nb = 2 if pools.get("kp_bufs", 1) > 1 else 1
                kp_t = pools["kp"].tile([128, 2, 256], FP16, name="kph",
                                        tag=f"kph{u}", bufs=nb)
                pools["dma_q"].dma_start(kp_t[:], kh_spill[h, :, b * 8 + u])
                kp8_t = pools["kp"].tile([128, 2, 2, 256], FP8, name="kp8",
                                         tag=f"kp8{u}", bufs=nb)
                pools["dma_q"].dma_start(kp8_t[:], k8_spill[h, :, b * 8 + u])
                kp_tiles[(b, h, u)] = (kp_t, kp8_t)

            def prefetch_qs(pools, b, h, u):
                qs_t = pools["qs"].tile([128, 2, 256], FP16, name="qsh",
                                        tag="qsh", bufs=2)
                pools["dma_q"].dma_start(qs_t[:], qh_spill[h, :, b * 8 + u])
                qs8_t = pools["qs"].tile([128, 2, 2, 256], FP8, name="qs8",
                                         tag="qs8", bufs=2)
                pools["dma_q"].dma_start(qs8_t[:], q8_spill[h, :, b * 8 + u])
                qs_tiles[(b, h, u)] = (qs_t, qs8_t)

            def produce(pools, b, h, u):
                """scores for super u -> softmax -> P^T via DMA transpose."""
                qs_t, qs8_t = qs_tiles.pop((b, h, u))
                nk = 2 * (u + 1)
                Lp = nk * 128
                pe_tr = "ps_pt" in pools
                if pe_tr:
                    pt_sb = pools["pt"].tile([128, SEQ // 128, 256], BF16,
                                             name="pt", tag="pt")
                else:
                    pt_sb = pools["pt"].tile([128, 2 * (SEQ // 128), 128],
                                             BF16, name="pt", tag="pt")
                pt_tiles[(b, h, u)] = (pt_sb, pe_tr)
                pstrip2 = pools["pstrip"].tile([128, 2 * SEQ], BF16,
                                               name="pstrip", tag="pstrip")
                for qt in range(2):
                    i = 2 * u + qt
                    strip = pools["strip"].tile([128, SEQ], F32,
                                                name="strip", tag="strip")
                    for j in range(u + 1):
                        ps_s = pools["ps_s"].tile([128, 256], F32,
                                                  name="ps_s", tag="ps_s")
                        kp_t, kp8_t = kp_tiles[(b, h, j)]
                        for ec in range(2):
                            nc.tensor.matmul(
                                ps_s[:],
                                qs_t[:, ec, qt * 128:(qt + 1) * 128],
                                kp_t[:, ec, :],
                                start=(ec == 0), stop=False)
                        for ec in range(2):
                            nc.tensor.matmul(
                                ps_s[:],
                                qs8_t[:, ec, :, qt * 128:(qt + 1) * 128],
                                kp8_t[:, ec, :, :],
                                perf_mode=mybir.MatmulPerfMode.DoubleRow,
                                start=False, stop=(ec == 1))
                        dst = strip[:, j * 256:(j + 1) * 256]
                        if j == u:
                            off = 128 if qt == 0 else 0
                            nc.vector.tensor_tensor(
                                dst, ps_s[:], mask_sb[:, off:off + 256],
                                mybir.AluOpType.add)
                        elif j % 2 == 0:
                            nc.scalar.copy(dst, ps_s[:])
                        else:
                            nc.vector.tensor_copy(dst, ps_s[:])
                    Lv = (i + 1) * 128
                    negmax = pools["stat"].tile([128, 1], F32, name="negmax",
                                                tag="negmax")
                    nc.vector.reduce_max(negmax[:], strip[:, :Lv],
                                         axis=mybir.AxisListType.X,
                                         negate=True)
                    pstrip = pstrip2[:, qt * Lp:(qt + 1) * Lp]
                    zsum = pools["stat"].tile([128, 1], F32, name="zsum",
                                              tag="zsum")
                    nc.scalar.activation(
                        pstrip, strip[:, :Lp],
                        mybir.ActivationFunctionType.Exp,
                        bias=negmax[:], scale=1.0, accum_out=zsum[:])
                    rz = pools["stat"].tile([128, 1], F32, name="rz",
                                            tag="rz")
                    nc.vector.reciprocal(rz[:], zsum[:])
                    nc.vector.tensor_scalar_mul(pstrip, pstrip, rz[:])
                if pe_tr:
                    for c in range(nk):
                        ps_pt = pools["ps_pt"].tile([128, 256], BF16,
                                                    name="ps_pt", tag="ps_pt")
                        for qt in range(2):
                            nc.tensor.transpose(
                                ps_pt[:, qt * 128:(qt + 1) * 128],
                                pstrip2[:, qt * Lp + c * 128:
                                        qt * Lp + (c + 1) * 128],
                                ident_holder["t"][:])
                        eng = nc.vector if c % 2 == 0 else nc.scalar
                        _copy(nc, eng, pt_sb[:, c, :], ps_pt[:])
                else:
                    pools["tq"].dma_start_transpose(
                        pt_sb[:, :2 * nk, :], pstrip2[:, :2 * Lp])

            def consume(pools, b, h, u):
                """PV for super u -> y^T chunk."""
                nk = 2 * (u + 1)
                pt_sb, pe_tr = pt_tiles.pop((b, h, u))
                ps_y = [pools["ps_y"].tile([128, 256], F32, name=f"ps_y{dh}",
                                           tag=f"ps_y{dh}")
                        for dh in range(2)]
                for c in range(nk):
                    g = b * (SEQ // 128) + c
                    rhs = pt_sb[:, c, :] if pe_tr \
                        else pt_sb[:, c:c + nk + 1:nk, :]
                    for dh in range(2):
                        nc.tensor.matmul(
                            ps_y[dh][:],
                            v_all[:, g, h * 256 + dh * 128:
                                  h * 256 + (dh + 1) * 128],
                            rhs,
                            start=(c == 0), stop=(c == nk - 1))
                yt = get_yt(b, u // 2)
                for dh in range(2):
                    eng = nc.vector if dh == 0 else nc.scalar
                    _copy(nc, eng,
                          yt[:, 2 * h + dh, (u % 2) * 256:(u % 2 + 1) * 256],
                          ps_y[dh][:])

            def proj_rs(pools, ci):
                """output projection for one RS chunk."""
                t0_tok, nt, b, parts = CHUNKS[ci]
                wp_sb = pools["wp_sb"]
                row = 0
                for (sl, st0, nst) in parts:
                    yt = yt_t[(b, sl)]
                    for st in range(st0, st0 + nst):
                        ost = pools["ost"].tile([128, OUT], BF16, name="ost",
                                                tag="ost")
                        for ob in range(OUT // 512):
                            ps_o = pools["ps_o"].tile([128, 512], F32,
                                                      name="ps_o", tag="ps_o")
                            for fc in range(4):
                                nc.tensor.matmul(
                                    ps_o[:],
                                    yt[:, fc, st * 128:(st + 1) * 128],
                                    wp_sb[:, fc, ob * 512:(ob + 1) * 512],
                                    start=(fc == 0), stop=(fc == 3))
                            eng = nc.vector if ob % 2 == 0 else nc.scalar
                            _copy(nc, eng, ost[:, ob * 512:(ob + 1) * 512],
                                  ps_o[:])
                        nc.sync.dma_start(partial_c[ci][row:row + 128, :],
                                          ost[:])
                        row += 128
                nc.gpsimd.collective_compute(
                    "ReduceScatter",
                    mybir.AluOpType.add,
                    ins=[partial_c[ci].opt()],
                    outs=[rs_out_c[ci].opt()],
                    replica_groups=[list(range(N_CORES))],
                )
                nc.gpsimd.dma_start(
                    out_ext.ap()[t0_tok // N_CORES:
                                 (t0_tok + nt) // N_CORES],
                    rs_out_c[ci])

            def drain_qk(stpool, pst, hidx, tb):
                """Drain 4 qk PSUM groups into fp16-hi + scaled-fp8 spills.
                q pieces: (lo*32, hi/32); k pieces: (hi/32, lo*32)."""
                for pair in range(2):
                    hi = stpool.tile([128, 2, 256], FP16, name="hi",
                                     tag="hi")
                    p8 = stpool.tile([128, 2, 2, 256], FP8, name="p8",
                                     tag="p8")
                    for half in range(2):
                        fc = pair * 2 + half
                        eng = nc.vector if half == 0 else nc.scalar
                        _copy(nc, eng, hi[:, half, :], pst[fc][:])
                        lo = stpool.tile([128, 256], FP16, name="lo",
                                         tag="lo")
                        nc.vector.tensor_tensor(lo[:], pst[fc][:],
                                                hi[:, half, :],
                                                mybir.AluOpType.subtract)
                        lo_pc = 0 if pair == 0 else 1
                        nc.gpsimd.tensor_scalar_mul(
                            p8[:, half, lo_pc, :], lo[:], LOSC)
                        nc.gpsimd.tensor_scalar_mul(
                            p8[:, half, 1 - lo_pc, :], hi[:, half, :],
                            1.0 / LOSC)
                    hdst = qh_spill if pair == 0 else kh_spill
                    dst8 = q8_spill if pair == 0 else k8_spill
                    nc.sync.dma_start(hdst[hidx, :, tb], hi[:])
                    nc.sync.dma_start(dst8[hidx, :, tb], p8[:])

            # ================= phase 1a: qk(h0) + v(both) =================
            with (
                nc.named_scope("proj_a"),
                tc.tile_pool(name="p1w", bufs=1) as p1w,
                tc.tile_pool(name="p1hs", bufs=3) as p1hs,
                tc.tile_pool(name="p1hb", bufs=3) as p1hb,
                tc.tile_pool(name="p1st", bufs=6) as p1st,
                tc.tile_pool(name="psA", bufs=1, space="PSUM") as psA,
                tc.tile_pool(name="psV", bufs=2, space="PSUM") as psV,
            ):
                wqk_sb = p1w.tile([128, ECH, 512], F32R, name="wqk0_sb")
                wv_sb = p1w.tile([128, ECH, 512], BF16, name="wv_sb")

                def load_wqk0(ci):
                    e0 = ci * (ECH // 4)
                    e1 = (ci + 1) * (ECH // 4)
                    nc.sync.dma_start(
                        wqk_sb[:, e0:e1, :],
                        wqk.ap()[e0:e1, :, 0:512].rearrange("ec p f -> p ec f"))
                    nc.scalar.dma_start(
                        wv_sb[:, e0:e1, :],
                        wv.ap()[e0:e1].rearrange("ec p f -> p ec f"))


                load_wqk0(0)
                for tb in range(NTB):
                    hs_eh = {}
                    hb_eh = {}

                    def load_group(g, tb=tb, hs_eh=hs_eh, hb_eh=hb_eh):
                        hs_t = p1hs.tile([128, 8, 256], F32R, name="hs_t",
                                         tag="hs_t")
                        nc.sync.dma_start(
                            hs_t[:],
                            hs5.ap()[tb, g * 8:(g + 1) * 8]
                            .rearrange("ec p t -> p ec t"))
                        hb_t = p1hb.tile([128, 8, 256], BF16, name="hb_t",
                                         tag="hb_t")
                        nc.gpsimd.tensor_copy(hb_t[:], hs_t[:])
                        hs_eh[g] = hs_t
                        hb_eh[g] = hb_t

                    load_group(0)
                    pst = [psA.tile([128, 256], F32, name=f"ps_qk{fc}",
                                    tag=f"ps_qk{fc}") for fc in range(4)]
                    ps_v = [psV.tile([128, 512], F32, name=f"ps_v{tc_i}",
                                     tag=f"ps_v{tc_i}") for tc_i in range(2)]

                    def emit_v(ec):
                        for tc_i in range(2):
                            nc.tensor.matmul(
                                ps_v[tc_i][:],
                                hb_eh[ec // 8][:, ec % 8,
                                               tc_i * 128:(tc_i + 1) * 128],
                                wv_sb[:, ec, :],
                                start=(ec == 0), stop=(ec == ECH - 1))

                    def emit_qk(ec):
                        for fc in range(4):
                            nc.tensor.matmul(
                                pst[fc][:],
                                wqk_sb[:, ec, fc * 128:(fc + 1) * 128],
                                hs_eh[ec // 8][:, ec % 8, :],
                                start=(ec == 0), stop=(ec == ECH - 1))

                    VLEAD = 3
                    for ec in range(VLEAD):
                        emit_v(ec)
                    for ec in range(ECH):
                        if ec % 8 == 0 and ec // 8 + 1 < 4:
                            load_group(ec // 8 + 1)
                            if tb == 0:
                                load_wqk0(ec // 8 + 1)
                        emit_qk(ec)
                        if ec + VLEAD < ECH:
                            emit_v(ec + VLEAD)
                    drain_qk(p1st, pst, 0, tb)
                    for tc_i in range(2):
                        eng = nc.scalar if tc_i == 0 else nc.vector
                        _copy(nc, eng, v_all[:, tb * 2 + tc_i, :],
                              ps_v[tc_i][:])

            # ====== phase 1b: qk(h1) || attn(b0), then attn(b1 lo) + rs ======
            import contextlib
            state = {"consumed": set(), "done": set()}

            def run_iters(pools, padded, proj_fn, lo, hi, do_chunks,
                          prefetch=True):
                for i in range(lo, hi):
                    if proj_fn is not None:
                        proj_fn(i)
                    s = padded[i] if i < len(padded) else None
                    if s is not None:
                        produce(pools, *s)
                    if prefetch:
                        nxt = padded[i + 2] if i + 2 < len(padded) else None
                        if nxt is not None:
                            prefetch_kp(pools, *nxt)
                    nq = padded[i + 1] if i + 1 < len(padded) else None
                    if nq is not None:
                        prefetch_qs(pools, *nq)
                    ps = padded[i - 3] if 3 <= i < len(padded) + 3 else None
                    if ps is not None:
                        consume(pools, *ps)
                        state["consumed"].add(ps)
                        if do_chunks:
                            for ci in range(len(CHUNKS)):
                                if ci not in state["done"] and \
                                        chunk_reqs(ci) <= state["consumed"]:
                                    state["done"].add(ci)
                                    proj_rs(pools, ci)

            with (
                nc.named_scope("proj_b_attn0"),
                contextlib.ExitStack() as stack,
            ):
                pools = make_attn_pools(stack, with_proj_out=False)
                pools["dma_q"] = nc.gpsimd
                pools["tq"] = nc.scalar

                padded = [(0, 0, u) for u in range(SUP)] + \
                         [(0, 1, u) for u in range(SUP)] + \
                         [None, None] + \
                         [(1, 0, u) for u in range(4)] + \
                         [(1, 1, u) for u in range(4)]

                with (
                    tc.tile_pool(name="p2w", bufs=1) as p2w,
                    tc.tile_pool(name="p2hs", bufs=2) as p2hs,
                    tc.tile_pool(name="p2st", bufs=2) as p2st,
                    tc.tile_pool(name="psB", bufs=1, space="PSUM") as psB,
                ):
                    wqk1_sb = p2w.tile([128, ECH, 512], F32R, name="wqk1_sb")

                    def load_wqk1(ci):
                        e0 = ci * (ECH // 4)
                        e1 = (ci + 1) * (ECH // 4)
                        nc.sync.dma_start(
                            wqk1_sb[:, e0:e1, :],
                            wqk.ap()[e0:e1, :, 512:1024]
                            .rearrange("ec p f -> p ec f"))

                    load_wqk1(0)

                    def proj_h1(tb):
                        hs_eh = {}

                        def load_group(g):
                            hs_t = p2hs.tile([128, 8, 256], F32R,
                                             name="hs2_t", tag="hs2_t")
                            nc.sync.dma_start(
                                hs_t[:],
                                hs5.ap()[tb, g * 8:(g + 1) * 8]
                                .rearrange("ec p t -> p ec t"))
                            hs_eh[g] = hs_t

                        load_group(0)
                        pst = [psB.tile([128, 256], F32, name=f"ps_qk1{fc}",
                                        tag=f"ps_qk1{fc}") for fc in range(4)]
                        for ec in range(ECH):
                            if ec % 8 == 0 and ec // 8 + 1 < 4:
                                load_group(ec // 8 + 1)
                                if tb == 0:
                                    load_wqk1(ec // 8 + 1)
                            for fc in range(4):
                                nc.tensor.matmul(
                                    pst[fc][:],
                                    wqk1_sb[:, ec, fc * 128:(fc + 1) * 128],
                                    hs_eh[ec // 8][:, ec % 8, :],
                                    start=(ec == 0), stop=(ec == ECH - 1))
                        drain_qk(p2st, pst, 1, tb)

                    def proj_h1_delayed(i):
                        if 2 <= i < NTB + 2:
                            proj_h1(i - 2)

                    for k in range(2):
                        prefetch_kp(pools, *padded[k])
                    prefetch_qs(pools, *padded[0])
                    run_iters(pools, padded, proj_h1_delayed, 0, NTB + 2,
                              do_chunks=False)

                # proj pools closed: b1 low supers + b0/b1-lo outproj + RS
                with contextlib.ExitStack() as stack2:
                    p2 = dict(pools)
                    add_proj_out_pools(stack2, p2)
                    p2["dma_q"] = nc.sync
                    run_iters(p2, padded, None, NTB + 2, len(padded) + 3,
                              do_chunks=True)
                kp_tiles.clear()

            # ============ phase 2b: attn(b1 hi) + proj + rs ============
            with (
                nc.named_scope("attn1"),
                contextlib.ExitStack() as stack,
            ):
                pools = make_attn_pools(stack, with_proj_out=True)
                pools["dma_q"] = nc.sync
                pools["tq"] = nc.sync
                pools["kp_bufs"] = 2
                stages_b1 = [(1, h, u) for u in range(4, SUP)
                             for h in range(2)]
                seen = set()
                for (b, h, u) in stages_b1:
                    for j in range(u + 1):
                        if (b, h, j) not in seen:
                            seen.add((b, h, j))
                            prefetch_kp(pools, b, h, j)
                prefetch_qs(pools, *stages_b1[0])
                run_iters(pools, stages_b1, None, 0, len(stages_b1) + 3,
                          do_chunks=True, prefetch=False)
                kp_tiles.clear()

    nc.finalize()
    return nc


def prep_inputs(cfg: Cfg, hidden_states, w_qkv, w_proj):
    """Shard + lay out the full inputs for each of the 8 cores."""
    import ml_dtypes
    seq, batch, e = hidden_states.shape
    assert (seq, batch, e) == (cfg.seq, cfg.batch, cfg.e)
    hs_t = np.ascontiguousarray(
        hidden_states.transpose(1, 0, 2).reshape(cfg.tok, e).T
    )  # [e, tok], tokens batch-major
    hs5 = np.ascontiguousarray(
        hs_t.reshape(cfg.ech, 128, cfg.ntb, 256).transpose(2, 0, 1, 3)
    ).astype(np.float32)

    scale = math.sqrt(math.sqrt(KV_CHANNELS))
    w3 = w_qkv.reshape(HEADS, 3, HD, e)
    mask = np.full((128, 1024), 0.0, dtype=np.float32)
    cols = np.arange(1024)[None, :]
    rows = np.arange(128)[:, None]
    mask[cols > 384 + rows] = NEG
    mask = np.ascontiguousarray(mask[:, 256:640])  # only cols 256..640 used
    ident = np.eye(128, dtype=ml_dtypes.bfloat16)

    in_maps = []
    for c in range(N_CORES):
        hsel = [2 * c, 2 * c + 1]
        # head-major qk: [h, (q(256), k(256))] -> [1024, e]
        wqk_rows = []
        for h in hsel:
            wqk_rows.append((w3[h, 0] * scale).reshape(HD, e))
            wqk_rows.append((w3[h, 1] * scale).reshape(HD, e))
        wqk_cat = np.concatenate(wqk_rows, axis=0)  # [1024, e] (q0,k0,q1,k1)
        # reorder to h*512 + (q,k): currently [q0(256),k0,q1,k1] == desired
        wqk_t = np.ascontiguousarray(wqk_cat.T.reshape(cfg.ech, 128, 1024))
        w_v = np.concatenate([w3[h, 2].reshape(HD, e) for h in hsel], axis=0)
        wv_t = np.ascontiguousarray(w_v.T.reshape(cfg.ech, 128, 512))
        wp_c = w_proj[:, c * 512:(c + 1) * 512]  # [OUT, 512]
        wp_t = np.ascontiguousarray(wp_c.T.reshape(4, 128, cfg.out))
        in_maps.append({
            "hs5": hs5,
            "wqk": wqk_t.astype(np.float32),
            "wv": wv_t.astype(ml_dtypes.bfloat16),
            "wp": wp_t.astype(ml_dtypes.bfloat16),
            "maskm": mask,
            "identm": ident,
        })
    return in_maps


def assemble_output(cfg: Cfg, results):
    """Gather per-core ReduceScatter shards into the full [seq, b, out]."""
    full = np.empty((cfg.tok, cfg.out), dtype=np.float32)
    for (t0, nt, _b, _parts) in CHUNKS:
        rows = nt // N_CORES
        for r in range(N_CORES):
            shard = results[r]["out_ext"][t0 // N_CORES:
                                          t0 // N_CORES + rows]
            full[t0 + r * rows:t0 + (r + 1) * rows] = \
                shard.astype(np.float32)
    return np.ascontiguousarray(
        full.reshape(cfg.batch, cfg.seq, cfg.out).transpose(1, 0, 2))


_NC_CACHE = {}


def run(cfg: Cfg, hidden_states, w_qkv, w_proj, trace=False):
    key = (cfg.seq, cfg.e, cfg.out)
    if key not in _NC_CACHE:
        _NC_CACHE[key] = build_kernel(cfg)
    nc = _NC_CACHE[key]
    in_maps = prep_inputs(cfg, hidden_states, w_qkv, w_proj)
    res = bass_utils.run_bass_kernel_spmd(
        nc, in_maps, core_ids=list(range(N_CORES)), trace=trace)
    return assemble_output(cfg, res.results), res


def kernel(hidden_states, attention_mask, w_qkv, w_proj):
    cfg = Cfg()
    out, _ = run(cfg, np.asarray(hidden_states, dtype=np.float32),
                 np.asarray(w_qkv, dtype=np.float32),
                 np.asarray(w_proj, dtype=np.float32))
    return out


# revision 46
# speedup vs baseline: 1.1645x; 1.0007x over previous
"""Trainium2 Bass kernel for nn_CausalSelfAttention_78331613544603.

Tensor-parallel over heads across 8 NeuronCores (Megatron-style).
Per core (2 heads), three software-pipelined phases:
  1a: QK projection for head0 + V projection for both heads (bf16).
  1b: QK projection for head1 on the PE, interleaved with causal
      attention for batch0 (both heads), batch0 output projection and
      its chunked ReduceScatter.
  2b: attention for batch1 + output projection + ReduceScatter.
Scores run in fp32r (1 cycle/row); P^T is produced by DMA transpose
(xbar) instead of PE transposes; partial sums reduce in bf16.

Self-contained: only needs numpy + the concourse toolchain staged at
/opt/trn_rl_repo (also importable via the environment's PYTHONPATH).
"""

import math
import sys

import numpy as np

try:
    import concourse.bass as bass
except ImportError:
    sys.path.insert(0, "/opt/trn_rl_repo")
    import concourse.bass as bass

import concourse.mybir as mybir
import concourse.tile as tile
from concourse import bacc, bass_utils

F32 = mybir.dt.float32
F32R = mybir.dt.float32r
BF16 = mybir.dt.bfloat16
FP16 = mybir.dt.float16
FP8 = mybir.dt.float8e4
LOSC = 32.0

N_CORES = 8
HEADS = 16
HPC = HEADS // N_CORES  # heads per core = 2
HD = 256  # head dim
KV_CHANNELS = 128
NEG = -1.0e30


def _copy(nc, eng, dst, src):
    if eng is nc.scalar:
        nc.scalar.copy(dst, src)
    else:
        eng.tensor_copy(dst, src)


# RS chunks: (token_start, n_tokens, batch, [(sl, st0, nst), ...]).
# b0 chunks run as phase-2b fillers; b1 chunks fire as their supers
# complete (2b processes supers in descending order, so high sls first;
# the last two chunks are 256 tokens to shrink the collective tail).
CHUNKS = [
    (0, 1024, 0, [(0, 0, 4), (1, 0, 4)]),
    (1024, 1024, 0, [(2, 0, 4), (3, 0, 4)]),
    (2048, 512, 1, [(0, 0, 4)]),
    (2560, 512, 1, [(1, 0, 4)]),
    (3072, 256, 1, [(2, 0, 2)]),
    (3328, 256, 1, [(2, 2, 2)]),
    (3584, 256, 1, [(3, 0, 2)]),
    (3840, 256, 1, [(3, 2, 2)]),
]


def chunk_reqs(ci):
    t0, nt, b, parts = CHUNKS[ci]
    req = set()
    for (sl, st0, nst) in parts:
        for st in range(st0, st0 + nst):
            u = 2 * sl + (1 if st >= 2 else 0)
            for h in range(2):
                req.add((b, h, u))
    return req


class Cfg:
    def __init__(self, seq=2048, e=4096, out=2048):
        self.seq = seq
        self.batch = 2
        self.e = e  # input embedding dim (2*HIDDEN)
        self.out = out  # output dim (HIDDEN)
        self.ech = e // 128  # contraction chunks (32)
        self.tok = seq * self.batch  # total tokens, batch-major (4096)
        self.ntb = self.tok // 256  # projection token blocks (16)
        self.supers = seq // 256  # q super-tiles per (b,h) (8)
        self.nstg = self.tok // 512  # reduce-scatter chunks (8)


def build_kernel(cfg: Cfg):
    nc = bacc.Bacc("TRN2", target_bir_lowering=False, debug=False,
                   num_devices=N_CORES)

    ECH = cfg.ech  # 32
    SEQ = cfg.seq
    TOK = cfg.tok
    OUT = cfg.out
    NTB = cfg.ntb
    SUP = cfg.supers

    # ---- kernel I/O ----
    hs5 = nc.dram_tensor("hs5", [NTB, ECH, 128, 256], F32R,
                         kind="ExternalInput")
    wqk = nc.dram_tensor("wqk", [ECH, 128, 1024], F32R,
                         kind="ExternalInput")  # head-major: h*512+(q,k)
    wv = nc.dram_tensor("wv", [ECH, 128, 512], BF16, kind="ExternalInput")
    wp = nc.dram_tensor("wp", [4, 128, OUT], BF16, kind="ExternalInput")
    maskm = nc.dram_tensor("maskm", [128, 384], F32, kind="ExternalInput")
    identm = nc.dram_tensor("identm", [128, 128], BF16, kind="ExternalInput")
    out_ext = nc.dram_tensor("out_ext", [cfg.tok // N_CORES, OUT],
                             BF16, kind="ExternalOutput")

    with tile.TileContext(nc) as tc:
        with (
            tc.tile_pool(name="const", bufs=1) as constp,
            tc.tile_pool(name="resident", bufs=1) as resp,
            tc.tile_pool(name="dram", bufs=1, space="DRAM") as dramp,
        ):
            mask_sb = constp.tile([128, 384], F32, name="mask_sb")
            nc.sync.dma_start(mask_sb[:], maskm.ap())
            ident_holder = {}

            # v for all tokens / both heads, resident through attention
            v_all = resp.tile([128, TOK // 128, 512], BF16, name="v_all")
            # y^T accumulators: 4 slots, reused across batches
            yt_t = {}

            def get_yt(b, sl):
                if (b, sl) not in yt_t:
                    yt_t[(b, sl)] = resp.tile(
                        [128, 4, 512], BF16, name=f"yt_{b}_{sl}",
                        tag=f"yt_{sl}")
                return yt_t[(b, sl)]

            qh_spill = dramp.tile([2, 128, NTB, 2, 256], FP16,
                                  name="qh_spill")
            kh_spill = dramp.tile([2, 128, NTB, 2, 256], FP16,
                                  name="kh_spill")
            q8_spill = dramp.tile([2, 128, NTB, 2, 2, 256], FP8,
                                  name="q8_spill")
            k8_spill = dramp.tile([2, 128, NTB, 2, 2, 256], FP8,
                                  name="k8_spill")
            partial_c = [dramp.tile([nt, OUT], BF16, name=f"partial{ci}",
                                    tag=f"partial{ci}")
                         for ci, (t0, nt, b, parts) in enumerate(CHUNKS)]
            rs_out_c = [dramp.tile([nt // N_CORES, OUT], BF16,
                                   name=f"rs_out{ci}", tag=f"rs_out{ci}")
                        for ci, (t0, nt, b, parts) in enumerate(CHUNKS)]

            # ====== shared attention machinery (used by 1b and 2b) ======
            def make_attn_pools(stack, with_proj_out):
                pools = {}
                pools["kp"] = stack.enter_context(
                    tc.tile_pool(name="kp", bufs=1))
                pools["qs"] = stack.enter_context(
                    tc.tile_pool(name="qs", bufs=3))
                pools["strip"] = stack.enter_context(
                    tc.tile_pool(name="strip", bufs=2))
                pools["pstrip"] = stack.enter_context(
                    tc.tile_pool(name="pstrip", bufs=2))
                pools["pt"] = stack.enter_context(
                    tc.tile_pool(name="pt", bufs=2))
                pools["stat"] = stack.enter_context(
                    tc.tile_pool(name="stat", bufs=2))
                pools["ps_s"] = stack.enter_context(
                    tc.tile_pool(name="ps_s", bufs=2, space="PSUM"))
                pools["ps_y"] = stack.enter_context(
                    tc.tile_pool(name="ps_y", bufs=1, space="PSUM"))
                if with_proj_out:
                    add_proj_out_pools(stack, pools)
                return pools

            def add_proj_out_pools(stack, pools):
                pools["ps_pt"] = stack.enter_context(
                    tc.tile_pool(name="ps_pt", bufs=2, space="PSUM"))
                pools["wpp"] = stack.enter_context(
                    tc.tile_pool(name="wpp", bufs=1))
                ident_sb = pools["wpp"].tile([128, 128], BF16,
                                             name="ident_sb")
                nc.sync.dma_start(ident_sb[:], identm.ap())
                ident_holder["t"] = ident_sb
                pools["ost"] = stack.enter_context(
                    tc.tile_pool(name="ost", bufs=2))
                pools["ps_o"] = stack.enter_context(
                    tc.tile_pool(name="ps_o", bufs=2, space="PSUM"))
                wp_sb = pools["wpp"].tile([128, 4, OUT], BF16, name="wp_sb")
                for ci in range(2):
                    nc.scalar.dma_start(
                        wp_sb[:, 2 * ci:2 * ci + 2, :],
                        wp.ap()[2 * ci:2 * ci + 2]
                        .rearrange("fc p o -> p fc o"))
                pools["wp_sb"] = wp_sb

            kp_tiles = {}
            qs_tiles = {}
            pt_tiles = {}

            def prefetch_kp(pools, b, h, u):
                """Load K increment j=u: fp16 hi + fp8 cross pieces."""
                nb = 2 if pools.get("kp_bufs", 1) > 1 else 1
                kp_t = pools["kp"].tile([128, 2, 256], FP16, name="kph",
                                        tag=f"kph{u}", bufs=nb)
                pools["dma_q"].dma_start(kp_t[:], kh_spill[h, :, b * 8 + u])
                kp8_t = pools["kp"].tile([128, 2, 2, 256], FP8, name="kp8",
                                         tag=f"kp8{u}", bufs=nb)
                pools["dma_q"].dma_start(kp8_t[:], k8_spill[h, :, b * 8 + u])
                kp_tiles[(b, h, u)] = (kp_t, kp8_t)

            def prefetch_qs(pools, b, h, u):
                qs_t = pools["qs"].tile([128, 2, 256], FP16, name="qsh",
                                        tag="qsh", bufs=2)
                pools["dma_q"].dma_start(qs_t[:], qh_spill[h, :, b * 8 + u])
                qs8_t = pools["qs"].tile([128, 2, 2, 256], FP8, name="qs8",
                                         tag="qs8", bufs=2)
                pools["dma_q"].dma_start(qs8_t[:], q8_spill[h, :, b * 8 + u])
                qs_tiles[(b, h, u)] = (qs_t, qs8_t)

            def produce(pools, b, h, u):
                """scores for super u -> softmax -> P^T via DMA transpose."""
                qs_t, qs8_t = qs_tiles.pop((b, h, u))
                nk = 2 * (u + 1)
                Lp = nk * 128
                pe_tr = "ps_pt" in pools
                if pe_tr:
                    pt_sb = pools["pt"].tile([128, SEQ // 128, 256], BF16,
                                             name="pt", tag="pt")
                else:
                    pt_sb = pools["pt"].tile([128, 2 * (SEQ // 128), 128],
                                             BF16, name="pt", tag="pt")
                pt_tiles[(b, h, u)] = (pt_sb, pe_tr)
                pstrip2 = pools["pstrip"].tile([128, 2 * SEQ], BF16,
                                               name="pstrip", tag="pstrip")
                for qt in range(2):
                    i = 2 * u + qt
                    strip = pools["strip"].tile([128, SEQ], F32,
                                                name="strip", tag="strip")
                    for j in range(u + 1):
                        ps_s = pools["ps_s"].tile([128, 256], F32,
                                                  name="ps_s", tag="ps_s")
                        kp_t, kp8_t = kp_tiles[(b, h, j)]
                        for ec in range(2):
                            nc.tensor.matmul(
                                ps_s[:],
                                qs_t[:, ec, qt * 128:(qt + 1) * 128],
                                kp_t[:, ec, :],
                                start=(ec == 0), stop=False)
                        for ec in range(2):
                            nc.tensor.matmul(
                                ps_s[:],
                                qs8_t[:, ec, :, qt * 128:(qt + 1) * 128],
                                kp8_t[:, ec, :, :],
                                perf_mode=mybir.MatmulPerfMode.DoubleRow,
                                start=False, stop=(ec == 1))
                        dst = strip[:, j * 256:(j + 1) * 256]
                        if j == u:
                            off = 128 if qt == 0 else 0
                            nc.vector.tensor_tensor(
                                dst, ps_s[:], mask_sb[:, off:off + 256],
                                mybir.AluOpType.add)
                        elif j % 2 == 0:
                            nc.scalar.copy(dst, ps_s[:])
                        else:
                            nc.vector.tensor_copy(dst, ps_s[:])
                    Lv = (i + 1) * 128
                    negmax = pools["stat"].tile([128, 1], F32, name="negmax",
                                                tag="negmax")
                    nc.vector.reduce_max(negmax[:], strip[:, :Lv],
                                         axis=mybir.AxisListType.X,
                                         negate=True)
                    pstrip = pstrip2[:, qt * Lp:(qt + 1) * Lp]
                    zsum = pools["stat"].tile([128, 1], F32, name="zsum",
                                              tag="zsum")
                    nc.scalar.activation(
                        pstrip, strip[:, :Lp],
                        mybir.ActivationFunctionType.Exp,
                        bias=negmax[:], scale=1.0, accum_out=zsum[:])
                    rz = pools["stat"].tile([128, 1], F32, name="rz",
                                            tag="rz")
                    nc.vector.reciprocal(rz[:], zsum[:])
                    nc.vector.tensor_scalar_mul(pstrip, pstrip, rz[:])
                if pe_tr:
                    for c in range(nk):
                        ps_pt = pools["ps_pt"].tile([128, 256], BF16,
                                                    name="ps_pt", tag="ps_pt")
                        for qt in range(2):
                            nc.tensor.transpose(
                                ps_pt[:, qt * 128:(qt + 1) * 128],
                                pstrip2[:, qt * Lp + c * 128:
                                        qt * Lp + (c + 1) * 128],
                                ident_holder["t"][:])
                        eng = nc.vector if c % 2 == 0 else nc.scalar
                        _copy(nc, eng, pt_sb[:, c, :], ps_pt[:])
                else:
                    pools["tq"].dma_start_transpose(
                        pt_sb[:, :2 * nk, :], pstrip2[:, :2 * Lp])

            def consume(pools, b, h, u):
                """PV for super u -> y^T chunk."""
                nk = 2 * (u + 1)
                pt_sb, pe_tr = pt_tiles.pop((b, h, u))
                ps_y = [pools["ps_y"].tile([128, 256], F32, name=f"ps_y{dh}",
                                           tag=f"ps_y{dh}")
                        for dh in range(2)]
                for c in range(nk):
                    g = b * (SEQ // 128) + c
                    rhs = pt_sb[:, c, :] if pe_tr \
                        else pt_sb[:, c:c + nk + 1:nk, :]
                    for dh in range(2):
                        nc.tensor.matmul(
                            ps_y[dh][:],
                            v_all[:, g, h * 256 + dh * 128:
                                  h * 256 + (dh + 1) * 128],
                            rhs,
                            start=(c == 0), stop=(c == nk - 1))
                yt = get_yt(b, u // 2)
                for dh in range(2):
                    eng = nc.vector if dh == 0 else nc.scalar
                    _copy(nc, eng,
                          yt[:, 2 * h + dh, (u % 2) * 256:(u % 2 + 1) * 256],
                          ps_y[dh][:])

            def proj_rs(pools, ci):
                """output projection for one RS chunk."""
                t0_tok, nt, b, parts = CHUNKS[ci]
                wp_sb = pools["wp_sb"]
                row = 0
                for (sl, st0, nst) in parts:
                    yt = yt_t[(b, sl)]
                    for st in range(st0, st0 + nst):
                        ost = pools["ost"].tile([128, OUT], BF16, name="ost",
                                                tag="ost")
                        for ob in range(OUT // 512):
                            ps_o = pools["ps_o"].tile([128, 512], F32,
                                                      name="ps_o", tag="ps_o")
                            for fc in range(4):
                                nc.tensor.matmul(
                                    ps_o[:],
                                    yt[:, fc, st * 128:(st + 1) * 128],
                                    wp_sb[:, fc, ob * 512:(ob + 1) * 512],
                                    start=(fc == 0), stop=(fc == 3))
                            eng = nc.vector if ob % 2 == 0 else nc.scalar
                            _copy(nc, eng, ost[:, ob * 512:(ob + 1) * 512],
                                  ps_o[:])
                        nc.sync.dma_start(partial_c[ci][row:row + 128, :],
                                          ost[:])
                        row += 128
                nc.gpsimd.collective_compute(
                    "ReduceScatter",
                    mybir.AluOpType.add,
                    ins=[partial_c[ci].opt()],
                    outs=[rs_out_c[ci].opt()],
                    replica_groups=[list(range(N_CORES))],
                )
                nc.gpsimd.dma_start(
                    out_ext.ap()[t0_tok // N_CORES:
                                 (t0_tok + nt) // N_CORES],
                    rs_out_c[ci])

            def drain_qk(stpool, pst, hidx, tb):
                """Drain 4 qk PSUM groups into fp16-hi + scaled-fp8 spills.
                q pieces: (lo*32, hi/32); k pieces: (hi/32, lo*32)."""
                for pair in range(2):
                    hi = stpool.tile([128, 2, 256], FP16, name="hi",
                                     tag="hi")
                    p8 = stpool.tile([128, 2, 2, 256], FP8, name="p8",
                                     tag="p8")
                    for half in range(2):
                        fc = pair * 2 + half
                        eng = nc.vector if half == 0 else nc.scalar
                        _copy(nc, eng, hi[:, half, :], pst[fc][:])
                        lo = stpool.tile([128, 256], FP16, name="lo",
                                         tag="lo")
                        nc.vector.tensor_tensor(lo[:], pst[fc][:],
                                                hi[:, half, :],
                                                mybir.AluOpType.subtract)
                        lo_pc = 0 if pair == 0 else 1
                        nc.gpsimd.tensor_scalar_mul(
                            p8[:, half, lo_pc, :], lo[:], LOSC)
                        nc.gpsimd.tensor_scalar_mul(
                            p8[:, half, 1 - lo_pc, :], hi[:, half, :],
                            1.0 / LOSC)
                    hdst = qh_spill if pair == 0 else kh_spill
                    dst8 = q8_spill if pair == 0 else k8_spill
                    nc.sync.dma_start(hdst[hidx, :, tb], hi[:])
                    nc.sync.dma_start(dst8[hidx, :, tb], p8[:])

            # ================= phase 1a: qk(h0) + v(both) =================
            with (
                nc.named_scope("proj_a"),
                tc.tile_pool(name="p1w", bufs=1) as p1w,
                tc.tile_pool(name="p1hs", bufs=3) as p1hs,
                tc.tile_pool(name="p1hb", bufs=3) as p1hb,
                tc.tile_pool(name="p1st", bufs=6) as p1st,
                tc.tile_pool(name="psA", bufs=1, space="PSUM") as psA,
                tc.tile_pool(name="psV", bufs=2, space="PSUM") as psV,
            ):
                wqk_sb = p1w.tile([128, ECH, 512], F32R, name="wqk0_sb")
                wv_sb = p1w.tile([128, ECH, 512], BF16, name="wv_sb")

                def load_wqk0(ci):
                    e0 = ci * (ECH // 4)
                    e1 = (ci + 1) * (ECH // 4)
                    nc.sync.dma_start(
                        wqk_sb[:, e0:e1, :],
                        wqk.ap()[e0:e1, :, 0:512].rearrange("ec p f -> p ec f"))
                    nc.scalar.dma_start(
                        wv_sb[:, e0:e1, :],
                        wv.ap()[e0:e1].rearrange("ec p f -> p ec f"))


                load_wqk0(0)
                for tb in range(NTB):
                    hs_eh = {}
                    hb_eh = {}

                    def load_group(g, tb=tb, hs_eh=hs_eh, hb_eh=hb_eh):
                        hs_t = p1hs.tile([128, 8, 256], F32R, name="hs_t",
                                         tag="hs_t")
                        nc.sync.dma_start(
                            hs_t[:],
                            hs5.ap()[tb, g * 8:(g + 1) * 8]
                            .rearrange("ec p t -> p ec t"))
                        hb_t = p1hb.tile([128, 8, 256], BF16, name="hb_t",
                                         tag="hb_t")
                        nc.gpsimd.tensor_copy(hb_t[:], hs_t[:])
                        hs_eh[g] = hs_t
                        hb_eh[g] = hb_t

                    load_group(0)
                    pst = [psA.tile([128, 256], F32, name=f"ps_qk{fc}",
                                    tag=f"ps_qk{fc}") for fc in range(4)]
                    ps_v = [psV.tile([128, 512], F32, name=f"ps_v{tc_i}",
                                     tag=f"ps_v{tc_i}") for tc_i in range(2)]

                    def emit_v(ec):
                        for tc_i in range(2):
                            nc.tensor.matmul(
                                ps_v[tc_i][:],
                                hb_eh[ec // 8][:, ec % 8,
                                               tc_i * 128:(tc_i + 1) * 128],
                                wv_sb[:, ec, :],
                                start=(ec == 0), stop=(ec == ECH - 1))

                    def emit_qk(ec):
                        for fc in range(4):
                            nc.tensor.matmul(
                                pst[fc][:],
                                wqk_sb[:, ec, fc * 128:(fc + 1) * 128],
                                hs_eh[ec // 8][:, ec % 8, :],
                                start=(ec == 0), stop=(ec == ECH - 1))

                    VLEAD = 3
                    for ec in range(VLEAD):
                        emit_v(ec)
                    for ec in range(ECH):
                        if ec % 8 == 0 and ec // 8 + 1 < 4:
                            load_group(ec // 8 + 1)
                            if tb == 0:
                                load_wqk0(ec // 8 + 1)
                        emit_qk(ec)
                        if ec + VLEAD < ECH:
                            emit_v(ec + VLEAD)
                    drain_qk(p1st, pst, 0, tb)
                    for tc_i in range(2):
                        eng = nc.scalar if tc_i == 0 else nc.vector
                        _copy(nc, eng, v_all[:, tb * 2 + tc_i, :],
                              ps_v[tc_i][:])

            # ====== phase 1b: qk(h1) || attn(b0), then attn(b1 lo) + rs ======
            import contextlib
            state = {"consumed": set(), "done": set()}

            def run_iters(pools, padded, proj_fn, lo, hi, do_chunks,
                          prefetch=True):
                for i in range(lo, hi):
                    if proj_fn is not None:
                        proj_fn(i)
                    s = padded[i] if i < len(padded) else None
                    if s is not None:
                        produce(pools, *s)
                    if prefetch:
                        nxt = padded[i + 2] if i + 2 < len(padded) else None
                        if nxt is not None:
                            prefetch_kp(pools, *nxt)
                    nq = padded[i + 1] if i + 1 < len(padded) else None
                    if nq is not None:
                        prefetch_qs(pools, *nq)
                    ps = padded[i - 3] if 3 <= i < len(padded) + 3 else None
                    if ps is not None:
                        consume(pools, *ps)
                        state["consumed"].add(ps)
                        if do_chunks:
                            for ci in range(len(CHUNKS)):
                                if ci not in state["done"] and \
                                        chunk_reqs(ci) <= state["consumed"]:
                                    state["done"].add(ci)
                                    proj_rs(pools, ci)

            with (
                nc.named_scope("proj_b_attn0"),
                contextlib.ExitStack() as stack,
            ):
                pools = make_attn_pools(stack, with_proj_out=False)
                pools["dma_q"] = nc.gpsimd
                pools["tq"] = nc.scalar

                padded = [(0, 0, u) for u in range(SUP)] + \
                         [(0, 1, u) for u in range(SUP)] + \
                         [None, None] + \
                         [(1, 0, u) for u in range(4)] + \
                         [(1, 1, u) for u in range(4)]

                with (
                    tc.tile_pool(name="p2w", bufs=1) as p2w,
                    tc.tile_pool(name="p2hs", bufs=2) as p2hs,
                    tc.tile_pool(name="p2st", bufs=2) as p2st,
                    tc.tile_pool(name="psB", bufs=1, space="PSUM") as psB,
                ):
                    wqk1_sb = p2w.tile([128, ECH, 512], F32R, name="wqk1_sb")

                    def load_wqk1(ci, split=1):
                        e0 = ci * (ECH // 4)
                        e1 = (ci + 1) * (ECH // 4)
                        step = (e1 - e0) // split
                        for s0 in range(e0, e1, step):
                            nc.sync.dma_start(
                                wqk1_sb[:, s0:s0 + step, :],
                                wqk.ap()[s0:s0 + step, :, 512:1024]
                                .rearrange("ec p f -> p ec f"))

                    load_wqk1(0, split=4)

                    def proj_h1(tb):
                        hs_eh = {}

                        def load_group(g):
                            hs_t = p2hs.tile([128, 8, 256], F32R,
                                             name="hs2_t", tag="hs2_t")
                            nc.sync.dma_start(
                                hs_t[:],
                                hs5.ap()[tb, g * 8:(g + 1) * 8]
                                .rearrange("ec p t -> p ec t"))
                            hs_eh[g] = hs_t

                        load_group(0)
                        pst = [psB.tile([128, 256], F32, name=f"ps_qk1{fc}",
                                        tag=f"ps_qk1{fc}") for fc in range(4)]
                        for ec in range(ECH):
                            if ec % 8 == 0 and ec // 8 + 1 < 4:
                                load_group(ec // 8 + 1)
                                if tb == 0:
                                    load_wqk1(ec // 8 + 1)
                            for fc in range(4):
                                nc.tensor.matmul(
                                    pst[fc][:],
                                    wqk1_sb[:, ec, fc * 128:(fc + 1) * 128],
                                    hs_eh[ec // 8][:, ec % 8, :],
                                    start=(ec == 0), stop=(ec == ECH - 1))
                        drain_qk(p2st, pst, 1, tb)

                    def proj_h1_delayed(i):
                        if 2 <= i < NTB + 2:
                            proj_h1(i - 2)

                    for k in range(2):
                        prefetch_kp(pools, *padded[k])
                    prefetch_qs(pools, *padded[0])
                    run_iters(pools, padded, proj_h1_delayed, 0, NTB + 2,
                              do_chunks=False)

                # proj pools closed: b1 low supers + b0/b1-lo outproj + RS
                with contextlib.ExitStack() as stack2:
                    p2 = dict(pools)
                    add_proj_out_pools(stack2, p2)
                    p2["dma_q"] = nc.sync
                    run_iters(p2, padded, None, NTB + 2, len(padded) + 3,
                              do_chunks=True)
                kp_tiles.clear()

            # ============ phase 2b: attn(b1 hi) + proj + rs ============
            with (
                nc.named_scope("attn1"),
                contextlib.ExitStack() as stack,
            ):
                pools = make_attn_pools(stack, with_proj_out=True)
                pools["dma_q"] = nc.sync
                pools["tq"] = nc.sync
                pools["kp_bufs"] = 2
                stages_b1 = [(1, h, u) for u in range(4, SUP)
                             for h in range(2)]
                seen = set()
                for (b, h, u) in stages_b1:
                    for j in range(u + 1):
                        if (b, h, j) not in seen:
                            seen.add((b, h, j))
                            prefetch_kp(pools, b, h, j)
                prefetch_qs(pools, *stages_b1[0])
                run_iters(pools, stages_b1, None, 0, len(stages_b1) + 3,
                          do_chunks=True, prefetch=False)
                kp_tiles.clear()

    nc.finalize()
    return nc


def prep_inputs(cfg: Cfg, hidden_states, w_qkv, w_proj):
    """Shard + lay out the full inputs for each of the 8 cores."""
    import ml_dtypes
    seq, batch, e = hidden_states.shape
    assert (seq, batch, e) == (cfg.seq, cfg.batch, cfg.e)
    hs_t = np.ascontiguousarray(
        hidden_states.transpose(1, 0, 2).reshape(cfg.tok, e).T
    )  # [e, tok], tokens batch-major
    hs5 = np.ascontiguousarray(
        hs_t.reshape(cfg.ech, 128, cfg.ntb, 256).transpose(2, 0, 1, 3)
    ).astype(np.float32)

    scale = math.sqrt(math.sqrt(KV_CHANNELS))
    w3 = w_qkv.reshape(HEADS, 3, HD, e)
    mask = np.full((128, 1024), 0.0, dtype=np.float32)
    cols = np.arange(1024)[None, :]
    rows = np.arange(128)[:, None]
    mask[cols > 384 + rows] = NEG
    mask = np.ascontiguousarray(mask[:, 256:640])  # only cols 256..640 used
    ident = np.eye(128, dtype=ml_dtypes.bfloat16)

    in_maps = []
    for c in range(N_CORES):
        hsel = [2 * c, 2 * c + 1]
        # head-major qk: [h, (q(256), k(256))] -> [1024, e]
        wqk_rows = []
        for h in hsel:
            wqk_rows.append((w3[h, 0] * scale).reshape(HD, e))
            wqk_rows.append((w3[h, 1] * scale).reshape(HD, e))
        wqk_cat = np.concatenate(wqk_rows, axis=0)  # [1024, e] (q0,k0,q1,k1)
        # reorder to h*512 + (q,k): currently [q0(256),k0,q1,k1] == desired
        wqk_t = np.ascontiguousarray(wqk_cat.T.reshape(cfg.ech, 128, 1024))
        w_v = np.concatenate([w3[h, 2].reshape(HD, e) for h in hsel], axis=0)
        wv_t = np.ascontiguousarray(w_v.T.reshape(cfg.ech, 128, 512))
        wp_c = w_proj[:, c * 512:(c + 1) * 512]  # [OUT, 512]
        wp_t = np.ascontiguousarray(wp_c.T.reshape(4, 128, cfg.out))
        in_maps.append({
            "hs5": hs5,
            "wqk": wqk_t.astype(np.float32),
            "wv": wv_t.astype(ml_dtypes.bfloat16),
            "wp": wp_t.astype(ml_dtypes.bfloat16),
            "maskm": mask,
            "identm": ident,
        })
    return in_maps


def assemble_output(cfg: Cfg, results):
    """Gather per-core ReduceScatter shards into the full [seq, b, out]."""
    full = np.empty((cfg.tok, cfg.out), dtype=np.float32)
    for (t0, nt, _b, _parts) in CHUNKS:
        rows = nt // N_CORES
        for r in range(N_CORES):
            shard = results[r]["out_ext"][t0 // N_CORES:
                                          t0 // N_CORES + rows]
            full[t0 + r * rows:t0 + (r + 1) * rows] = \
                shard.astype(np.float32)
    return np.ascontiguousarray(
        full.reshape(cfg.batch, cfg.seq, cfg.out).transpose(1, 0, 2))


_NC_CACHE = {}


def run(cfg: Cfg, hidden_states, w_qkv, w_proj, trace=False):
    key = (cfg.seq, cfg.e, cfg.out)
    if key not in _NC_CACHE:
        _NC_CACHE[key] = build_kernel(cfg)
    nc = _NC_CACHE[key]
    in_maps = prep_inputs(cfg, hidden_states, w_qkv, w_proj)
    res = bass_utils.run_bass_kernel_spmd(
        nc, in_maps, core_ids=list(range(N_CORES)), trace=trace)
    return assemble_output(cfg, res.results), res


def kernel(hidden_states, attention_mask, w_qkv, w_proj):
    cfg = Cfg()
    out, _ = run(cfg, np.asarray(hidden_states, dtype=np.float32),
                 np.asarray(w_qkv, dtype=np.float32),
                 np.asarray(w_proj, dtype=np.float32))
    return out
